# revision 11
# baseline (speedup 1.0000x reference)
"""Trainium2 Bass kernel for the sparse BasicBlock problem.

Math (exploiting binary masks / nonneg vector from setup_inputs):
    g   = x * mask_dilate
    c1  = conv3x3(g, w1)
    h   = relu(c1 * s1v + t1v) * mask          (s1v/t1v = BN1 affine folded with `vector`)
    c2  = conv3x3(h, w2)
    out = relu(x + (c2 * s2 + t2) * mask)

Layout: per image, channels on SBUF partitions (2 tiles of 128), spatial flattened
as a zero-padded (H+2)x(W+2) row-major plane so a 3x3 conv is 9 shifted matmuls
accumulating in PSUM (shift = (dy-1)*(W+2) + (dx-1)).

Sharding: data-parallel over batch, 4 images per core on 8 cores; conv/bn params
replicated (hint followed).
"""

import sys
import types

sys.path.insert(0, "/opt/trn_rl_repo")

import numpy as np

import concourse.bacc as bacc
import concourse.bass as bass
import concourse.mybir as mybir
import concourse.tile as tile
from concourse import bass_utils

# ----------------------------------------------------------------------------
# axon NTFF profiling hook shim (enables trace=True under axon); harmless if
# profiling is never requested.
# ----------------------------------------------------------------------------
_HOOK = {"hook": None}


def _install_axon_hooks():
    try:
        import antenv  # noqa: F401
    except ImportError:
        return
    if "antenv.axon_hooks" not in sys.modules:
        mod = types.ModuleType("antenv.axon_hooks")
        mod.set_axon_ntff_profile_hook = lambda h: _HOOK.__setitem__("hook", h)
        mod.get_axon_ntff_profile_hook = lambda: _HOOK["hook"]
        sys.modules["antenv.axon_hooks"] = mod
    if _HOOK["hook"] is None:
        try:
            from trn_agent_boot.trn_boot import _ntff_profile_via_ctypes

            sys.modules["antenv.axon_hooks"].set_axon_ntff_profile_hook(
                _ntff_profile_via_ctypes("/opt/axon/libaxon_pjrt.so")
            )
        except Exception:
            pass


_install_axon_hooks()
bass_utils.upload_artifacts = lambda tmpdir: tmpdir  # no S3 in this container

# ----------------------------------------------------------------------------
# problem constants (hardcoded per spec)
# ----------------------------------------------------------------------------
B, C, H, W = 32, 256, 56, 56
NCORES = 8
BPC = B // NCORES  # images per core
EPS = 1e-5

# runtime knobs (test.py may flip these)
TRACE = False
MM_MODE = "f32r"  # 'f32r' | 'bf16' | 'f32'
LAST_EXEC_NS = None
LAST_TRACE = None

F32 = mybir.dt.float32


def _chunks(total, maxw):
    """Split `total` columns into EVEN-width chunks of <= maxw, all >= 256 when
    possible (fp32r matmul requires an even moving dim and runs at full PE rate
    only for moving dim >= 256)."""
    assert total % 2 == 0, total
    n = -(-total // maxw)
    base = (total // n) & ~1
    rem = total - base * n
    assert rem % 2 == 0
    out = []
    off = 0
    for i in range(n):
        w = base + (2 if i < rem // 2 else 0)
        out.append((off, w))
        off += w
    assert off == total
    return out


def build_nc(mm_mode=MM_MODE, bpc=BPC, c=C, h=H, w=W):
    """Build the per-core Bass program (SPMD: same program, per-core shards)."""
    PW = w + 2  # padded width
    PH = h + 2
    FLAT = PH * PW
    CT = c // 128  # channel tiles
    NS = 9  # 3x3 shifts
    shifts = [(dy - 1) * PW + (dx - 1) for dy in range(3) for dx in range(3)]
    # output positions: padded rows 1..h, full PW width, minus the two ends so
    # that every shifted read stays inside [0, FLAT)
    out_lo = PW + 1  # (row1, col1)
    out_hi = h * PW + w  # (row h, col w) inclusive
    span = out_hi - out_lo + 1
    chunks = [(out_lo + o, s) for (o, s) in _chunks(span, 464)]
    chunk_alloc = max(s for _, s in chunks)

    # dtype of matmul operands (g/h activations + weights); fp32r tensors must be
    # *produced* as fp32r (the verifier enforces rounded producers)
    mdt = {"f32r": mybir.dt.float32r, "bf16": mybir.dt.bfloat16, "f32": F32}[mm_mode]
    # dtype of the conv1 ACT output (input to the h mask-multiply)
    edt = F32 if mm_mode == "f32r" else mdt
    mm_cast = lambda ap: ap  # noqa: E731

    nc = bacc.Bacc("TRN2", debug=False, enable_asserts=False, num_devices=NCORES)

    BF16 = mybir.dt.bfloat16
    x_d = nc.dram_tensor("x", [bpc, c, h * w], F32, kind="ExternalInput").ap()
    mask_d = nc.dram_tensor("mask", [bpc, h * w], BF16, kind="ExternalInput").ap()
    maskd_d = nc.dram_tensor("maskd", [bpc, h * w], BF16, kind="ExternalInput").ap()
    w1_d = nc.dram_tensor("w1", [CT, 128, NS, c], mdt, kind="ExternalInput").ap()
    w2_d = nc.dram_tensor("w2", [CT, 128, NS, c], mdt, kind="ExternalInput").ap()
    s1v_d = nc.dram_tensor("s1v", [bpc, c, 1], F32, kind="ExternalInput").ap()
    t1v_d = nc.dram_tensor("t1v", [bpc, c, 1], F32, kind="ExternalInput").ap()
    s2_d = nc.dram_tensor("s2", [c, 1], F32, kind="ExternalInput").ap()
    t2_d = nc.dram_tensor("t2", [c, 1], F32, kind="ExternalInput").ap()
    out_d = nc.dram_tensor("out", [bpc, c, h * w], F32, kind="ExternalOutput").ap()

    def bcast(ap_1d, rows=128):
        # DMA-side partition broadcast: read the same DRAM row into each partition
        return bass.AP(
            tensor=ap_1d.tensor,
            offset=ap_1d.offset,
            ap=[[0, rows], [w, h], [1, w]],
        )

    Relu = mybir.ActivationFunctionType.Relu
    Ident = mybir.ActivationFunctionType.Identity

    with tile.TileContext(nc) as tc:
        from contextlib import ExitStack

        with ExitStack() as ctx:
            wpool = ctx.enter_context(tc.tile_pool(name="wpool", bufs=1))
            cpool = ctx.enter_context(tc.tile_pool(name="cpool", bufs=1))
            ppool = ctx.enter_context(tc.tile_pool(name="ppool", bufs=2 * CT))
            xpool = ctx.enter_context(tc.tile_pool(name="xpool", bufs=CT + 1))
            spool = ctx.enter_context(tc.tile_pool(name="spool", bufs=CT + 1))
            hpool = ctx.enter_context(tc.tile_pool(name="hpool", bufs=CT))
            mpool = ctx.enter_context(tc.tile_pool(name="mpool", bufs=2))
            mdpool = ctx.enter_context(tc.tile_pool(name="mdpool", bufs=2))
            epool = ctx.enter_context(tc.tile_pool(name="epool", bufs=6))
            pspool = ctx.enter_context(tc.tile_pool(name="psum", bufs=6, space="PSUM"))

            # ---- persistent weights & bn2 params ----
            w1_sb = wpool.tile([128, CT, NS, c], mdt)
            w2_sb = wpool.tile([128, CT, NS, c], mdt)
            for ci_t in range(CT):
                nc.sync.dma_start(out=w1_sb[:, ci_t], in_=w1_d[ci_t])
                nc.sync.dma_start(out=w2_sb[:, ci_t], in_=w2_d[ci_t])
            s2_sb = cpool.tile([128, CT, 1], F32)
            t2_sb = cpool.tile([128, CT, 1], F32)
            for co_t in range(CT):
                nc.sync.dma_start(out=s2_sb[:, co_t], in_=s2_d[co_t * 128 : (co_t + 1) * 128])
                nc.sync.dma_start(out=t2_sb[:, co_t], in_=t2_d[co_t * 128 : (co_t + 1) * 128])

            def conv(w_sb, rhs_tiles, epilogue):
                """9-shift conv; epilogue(co_t, off, wd, ps_ap) consumes PSUM."""
                for co_t in range(CT):
                    for off, wd in chunks:
                        ps = pspool.tile([128, chunk_alloc], F32)
                        k = 0
                        nk = CT * NS
                        for ci_t in range(CT):
                            for s in range(NS):
                                nc.tensor.matmul(
                                    ps[:, :wd],
                                    mm_cast(w_sb[:, ci_t, s, co_t * 128 : co_t * 128 + 128]),
                                    mm_cast(
                                        rhs_tiles[ci_t][:, off + shifts[s] : off + shifts[s] + wd]
                                    ),
                                    start=(k == 0),
                                    stop=(k == nk - 1),
                                )
                                k += 1
                        epilogue(co_t, off, wd, ps)

            for i in range(bpc):
                # ---- masks (broadcast across partitions, zero-padded) ----
                mask_pad = mpool.tile([128, FLAT], BF16)
                m3 = mask_pad.rearrange("p (a b) -> p a b", b=PW)
                nc.vector.memset(m3[:, 1 : h + 1, 0:1], 0.0)
                nc.vector.memset(m3[:, 1 : h + 1, w + 1 : w + 2], 0.0)
                nc.sync.dma_start(out=m3[:, 1 : h + 1, 1 : w + 1], in_=bcast(mask_d[i]))

                maskd_pad = mdpool.tile([128, FLAT], BF16)
                md3 = maskd_pad.rearrange("p (a b) -> p a b", b=PW)
                nc.vector.memset(md3[:, 0, :], 0.0)
                nc.vector.memset(md3[:, h + 1, :], 0.0)
                nc.vector.memset(md3[:, 1 : h + 1, 0:1], 0.0)
                nc.vector.memset(md3[:, 1 : h + 1, w + 1 : w + 2], 0.0)
                nc.sync.dma_start(out=md3[:, 1 : h + 1, 1 : w + 1], in_=bcast(maskd_d[i]))

                # ---- x (padded) and g = x * mask_dilate ----
                x_pad = []
                g_pad = []
                for ci_t in range(CT):
                    xt = xpool.tile([128, FLAT], F32, tag="x")
                    x3 = xt.rearrange("p (a b) -> p a b", b=PW)
                    nc.vector.memset(x3[:, 0, :], 0.0)
                    nc.vector.memset(x3[:, h + 1, :], 0.0)
                    nc.vector.memset(x3[:, 1 : h + 1, 0:1], 0.0)
                    nc.vector.memset(x3[:, 1 : h + 1, w + 1 : w + 2], 0.0)
                    nc.sync.dma_start(
                        out=x3[:, 1 : h + 1, 1 : w + 1],
                        in_=x_d[i, ci_t * 128 : (ci_t + 1) * 128].rearrange(
                            "p (a b) -> p a b", b=w
                        ),
                    )
                    x_pad.append(xt)
                    gt = spool.tile([128, FLAT], mdt, tag="scr")
                    nc.vector.tensor_mul(gt, xt, maskd_pad)
                    g_pad.append(gt)

                # ---- per-image folded bn1*vector params ----
                s1v_t, t1v_t = [], []
                for co_t in range(CT):
                    st = ppool.tile([128, 1], F32, tag="s1v")
                    tt = ppool.tile([128, 1], F32, tag="t1v")
                    nc.sync.dma_start(out=st, in_=s1v_d[i, co_t * 128 : (co_t + 1) * 128])
                    nc.sync.dma_start(out=tt, in_=t1v_d[i, co_t * 128 : (co_t + 1) * 128])
                    s1v_t.append(st)
                    t1v_t.append(tt)

                # ---- conv1 + epilogue -> h ----
                h_pad = []
                for co_t in range(CT):
                    ht = hpool.tile([128, FLAT], mdt, tag="h")
                    # zero the padded ends via x0.0 (memset can't produce fp32r)
                    nc.vector.tensor_scalar_mul(
                        ht[:, 0:out_lo], x_pad[co_t][:, 0:out_lo], 0.0
                    )
                    nc.vector.tensor_scalar_mul(
                        ht[:, out_hi + 1 : FLAT], x_pad[co_t][:, out_hi + 1 : FLAT], 0.0
                    )
                    h_pad.append(ht)

                def epi1(co_t, off, wd, ps):
                    r = epool.tile([128, chunk_alloc], edt, tag="e")
                    nc.scalar.activation(
                        r[:, :wd], ps[:, :wd], Relu, bias=t1v_t[co_t], scale=s1v_t[co_t]
                    )
                    nc.vector.tensor_mul(
                        h_pad[co_t][:, off : off + wd], r[:, :wd], mask_pad[:, off : off + wd]
                    )

                conv(w1_sb, g_pad, epi1)

                # ---- conv2 + epilogue -> out ----
                out_t = [
                    spool.tile([128, FLAT], F32, tag="scr", name=f"out_t{i}_{ct}")
                    for ct in range(CT)
                ]

                def epi2(co_t, off, wd, ps):
                    e = epool.tile([128, chunk_alloc], F32, tag="e")
                    nc.scalar.activation(
                        e[:, :wd],
                        ps[:, :wd],
                        Ident,
                        bias=t2_sb[:, co_t],
                        scale=s2_sb[:, co_t],
                    )
                    nc.vector.tensor_mul(e[:, :wd], e[:, :wd], mask_pad[:, off : off + wd])
                    dst = out_t[co_t][:, off : off + wd]
                    nc.vector.tensor_add(dst, e[:, :wd], x_pad[co_t][:, off : off + wd])
                    nc.scalar.activation(dst, dst, Relu)

                conv(w2_sb, h_pad, epi2)

                for co_t in range(CT):
                    o3 = out_t[co_t].rearrange("p (a b) -> p a b", b=PW)
                    nc.sync.dma_start(
                        out=out_d[i, co_t * 128 : (co_t + 1) * 128].rearrange(
                            "p (a b) -> p a b", b=w
                        ),
                        in_=o3[:, 1 : h + 1, 1 : w + 1],
                    )

    nc.compile()
    return nc


# ----------------------------------------------------------------------------
# host-side prep + execution
# ----------------------------------------------------------------------------
_NC_CACHE = {}


def _prep_weights(wt, mm_mode, c=C):
    # [co, ci, 3, 3] -> [ci_t, ci, s, co] with s = dy*3+dx
    t = np.ascontiguousarray(wt.transpose(1, 2, 3, 0).reshape(c // 128, 128, 9, c))
    if mm_mode == "bf16":
        import ml_dtypes

        t = t.astype(ml_dtypes.bfloat16)
    return t


def kernel(**inputs):
    global LAST_EXEC_NS, LAST_TRACE
    x = np.asarray(inputs["x"], dtype=np.float32)
    import ml_dtypes

    mask = np.asarray(inputs["mask"], dtype=np.float32).reshape(B, H * W)
    maskd = np.asarray(inputs["mask_dilate"], dtype=np.float32).reshape(B, H * W)
    mask_bf = mask.astype(ml_dtypes.bfloat16)
    maskd_bf = maskd.astype(ml_dtypes.bfloat16)
    vector = np.asarray(inputs["vector"], dtype=np.float32)

    # folded BN params (parameter preprocessing only)
    s1 = np.asarray(inputs["bn1_g"]) / np.sqrt(np.asarray(inputs["bn1_v"]) + EPS)
    t1 = np.asarray(inputs["bn1_b"]) - np.asarray(inputs["bn1_m"]) * s1
    s2 = np.asarray(inputs["bn2_g"]) / np.sqrt(np.asarray(inputs["bn2_v"]) + EPS)
    t2 = np.asarray(inputs["bn2_b"]) - np.asarray(inputs["bn2_m"]) * s2
    # fold `vector` into bn1 affine: relu(z*v) == relu(z)*v for v >= 0
    assert (vector >= 0).all(), "vector must be nonnegative for the folded epilogue"
    s1v = (s1[None, :] * vector).astype(np.float32)  # [B, C]
    t1v = (t1[None, :] * vector).astype(np.float32)

    mm_mode = MM_MODE
    if mm_mode not in _NC_CACHE:
        _NC_CACHE[mm_mode] = build_nc(mm_mode)
    nc = _NC_CACHE[mm_mode]

    w1l = _prep_weights(np.asarray(inputs["conv1_w"], dtype=np.float32), mm_mode)
    w2l = _prep_weights(np.asarray(inputs["conv2_w"], dtype=np.float32), mm_mode)

    xs = x.reshape(NCORES, BPC, C, H * W)
    in_maps = []
    for cid in range(NCORES):
        sl = slice(cid * BPC, (cid + 1) * BPC)
        in_maps.append(
            dict(
                x=np.ascontiguousarray(xs[cid]),
                mask=np.ascontiguousarray(mask_bf[sl]),
                maskd=np.ascontiguousarray(maskd_bf[sl]),
                w1=w1l,
                w2=w2l,
                s1v=np.ascontiguousarray(s1v[sl].reshape(BPC, C, 1)),
                t1v=np.ascontiguousarray(t1v[sl].reshape(BPC, C, 1)),
                s2=np.ascontiguousarray(s2.reshape(C, 1).astype(np.float32)),
                t2=np.ascontiguousarray(t2.reshape(C, 1).astype(np.float32)),
            )
        )

    res = bass_utils.run_bass_kernel_spmd(
        nc, in_maps, core_ids=list(range(NCORES)), trace=TRACE
    )
    LAST_EXEC_NS = res.exec_time_ns
    LAST_TRACE = res.instructions_and_trace[1] if res.instructions_and_trace else None

    out = np.concatenate([r["out"][None] for r in res.results], axis=0)
    return out.reshape(B, C, H, W)


# revision 13
# speedup vs baseline: 1.2009x; 1.2009x over previous
"""Trainium2 Bass kernel for the sparse BasicBlock problem.

Math (masks and `vector` are binary in setup_inputs; verified at runtime):
    g   = x * mask_dilate
    c1  = conv3x3(g, w1)
    h   = relu(c1 * s1v + t1v) * mask      (BN1 affine folded with `vector`)
    c2  = conv3x3(h, w2)
    out = relu(x + (c2 * s2 + t2) * mask)

Layout: per image, channels on SBUF partitions, spatial flattened as a
zero-padded (H+2)x(W+2) row-major plane so a 3x3 conv is 9 shifted matmuls
accumulating in PSUM (shift = (dy-1)*(W+2) + (dx-1)). Matmuls run in fp32r
(full PE rate, ~1e-4 relative error).

Channel sparsity: `vector` zeroes ~half of conv1's output channels per image
(h == 0 there), so conv1 computes only the active channels (M-compaction) and
conv2 contracts only over them (K-compaction), via host-side gathered and
zero-padded per-image weights. One SPMD program is shared by all 8 cores, so
images are sorted by active-channel count and assigned so that each image
slot has a fixed channel-tile count across cores (max over cores).

Sharding: data-parallel over batch, 4 images per core on 8 cores.
"""

import sys
import types
from contextlib import ExitStack

sys.path.insert(0, "/opt/trn_rl_repo")

import ml_dtypes
import numpy as np

import concourse.bacc as bacc
import concourse.bass as bass
import concourse.mybir as mybir
import concourse.tile as tile
from concourse import bass_utils

# ----------------------------------------------------------------------------
# axon NTFF profiling hook shim (enables trace=True under axon)
# ----------------------------------------------------------------------------
_HOOK = {"hook": None}


def _install_axon_hooks():
    try:
        import antenv  # noqa: F401
    except ImportError:
        return
    if "antenv.axon_hooks" not in sys.modules:
        mod = types.ModuleType("antenv.axon_hooks")
        mod.set_axon_ntff_profile_hook = lambda h: _HOOK.__setitem__("hook", h)
        mod.get_axon_ntff_profile_hook = lambda: _HOOK["hook"]
        sys.modules["antenv.axon_hooks"] = mod
    if _HOOK["hook"] is None:
        try:
            from trn_agent_boot.trn_boot import _ntff_profile_via_ctypes

            sys.modules["antenv.axon_hooks"].set_axon_ntff_profile_hook(
                _ntff_profile_via_ctypes("/opt/axon/libaxon_pjrt.so")
            )
        except Exception:
            pass


_install_axon_hooks()
bass_utils.upload_artifacts = lambda tmpdir: tmpdir  # no S3 in this container

# ----------------------------------------------------------------------------
# problem constants (hardcoded per spec)
# ----------------------------------------------------------------------------
B, C, H, W = 32, 256, 56, 56
NCORES = 8
BPC = B // NCORES
EPS = 1e-5

TRACE = False
MM_MODE = "f32r"  # 'f32r' | 'bf16' | 'f32'
SPARSE = True
LAST_EXEC_NS = None
LAST_TRACE = None

F32 = mybir.dt.float32
BF16 = mybir.dt.bfloat16


def _chunks(total, maxw):
    """EVEN-width chunks <= maxw (fp32r needs an even moving dim; >=256 keeps
    full PE rate)."""
    assert total % 2 == 0, total
    n = -(-total // maxw)
    base = (total // n) & ~1
    rem = total - base * n
    out = []
    off = 0
    for i in range(n):
        w = base + (2 if i < rem // 2 else 0)
        out.append((off, w))
        off += w
    assert off == total
    return out


def _mdt(mm_mode):
    return {"f32r": mybir.dt.float32r, "bf16": BF16, "f32": F32}[mm_mode]


def build_nc(mm_mode=MM_MODE, bpc=BPC, c=C, h=H, w=W, slot_tiles=None):
    """Build the per-core SPMD Bass program.

    slot_tiles: None for the dense kernel, else per-image-slot channel-tile
    counts (e.g. (2, 2, 1, 1)) for the sparsity-specialized kernel.
    """
    PW, PH = w + 2, h + 2
    FLAT = PH * PW
    CT = c // 128
    NS = 9
    shifts = [(dy - 1) * PW + (dx - 1) for dy in range(3) for dx in range(3)]
    out_lo = PW + 1
    out_hi = h * PW + w
    span = out_hi - out_lo + 1
    chunks = [(out_lo + o, s) for (o, s) in _chunks(span, 464)]
    chunk_alloc = max(s for _, s in chunks)

    sparse = slot_tiles is not None
    if sparse:
        assert len(slot_tiles) == bpc
        max_nt = max(slot_tiles)
    mdt = _mdt(mm_mode)
    edt = F32 if mm_mode == "f32r" else mdt

    nc = bacc.Bacc("TRN2", debug=False, enable_asserts=False, num_devices=NCORES)

    x_d = nc.dram_tensor("x", [bpc, c, h * w], F32, kind="ExternalInput").ap()
    mask_d = nc.dram_tensor("mask", [bpc, h * w], BF16, kind="ExternalInput").ap()
    maskd_d = nc.dram_tensor("maskd", [bpc, h * w], BF16, kind="ExternalInput").ap()
    s2_d = nc.dram_tensor("s2", [c, 1], F32, kind="ExternalInput").ap()
    t2_d = nc.dram_tensor("t2", [c, 1], F32, kind="ExternalInput").ap()
    out_d = nc.dram_tensor("out", [bpc, c, h * w], F32, kind="ExternalOutput").ap()
    if sparse:
        w1_d, w2_d, s1_d, t1_d = [], [], [], []
        for s, nt in enumerate(slot_tiles):
            np_s = 128 * nt
            w1_d.append(
                nc.dram_tensor(f"w1g{s}", [CT, 128, NS, np_s], mdt, kind="ExternalInput").ap()
            )
            w2_d.append(
                nc.dram_tensor(f"w2g{s}", [nt, 128, NS, c], mdt, kind="ExternalInput").ap()
            )
            s1_d.append(
                nc.dram_tensor(f"s1vg{s}", [np_s, 1], F32, kind="ExternalInput").ap()
            )
            t1_d.append(
                nc.dram_tensor(f"t1vg{s}", [np_s, 1], F32, kind="ExternalInput").ap()
            )
    else:
        w1s_d = nc.dram_tensor("w1", [CT, 128, NS, c], mdt, kind="ExternalInput").ap()
        w2s_d = nc.dram_tensor("w2", [CT, 128, NS, c], mdt, kind="ExternalInput").ap()
        s1v_d = nc.dram_tensor("s1v", [bpc, c, 1], F32, kind="ExternalInput").ap()
        t1v_d = nc.dram_tensor("t1v", [bpc, c, 1], F32, kind="ExternalInput").ap()

    def bcast(ap_1d, rows=128):
        return bass.AP(
            tensor=ap_1d.tensor,
            offset=ap_1d.offset,
            ap=[[0, rows], [w, h], [1, w]],
        )

    Relu = mybir.ActivationFunctionType.Relu
    Ident = mybir.ActivationFunctionType.Identity

    with tile.TileContext(nc) as tc, ExitStack() as ctx:
        wpool = ctx.enter_context(tc.tile_pool(name="wpool", bufs=1))
        cpool = ctx.enter_context(tc.tile_pool(name="cpool", bufs=1))
        ppool = ctx.enter_context(tc.tile_pool(name="ppool", bufs=2))
        xpool = ctx.enter_context(tc.tile_pool(name="xpool", bufs=CT + 1))
        spool = ctx.enter_context(tc.tile_pool(name="spool", bufs=CT + 1))
        hpool = ctx.enter_context(
            tc.tile_pool(name="hpool", bufs=(max(2, max_nt) if sparse else CT))
        )
        mpool = ctx.enter_context(tc.tile_pool(name="mpool", bufs=2))
        mdpool = ctx.enter_context(tc.tile_pool(name="mdpool", bufs=2))
        epool = ctx.enter_context(tc.tile_pool(name="epool", bufs=6))
        pspool = ctx.enter_context(tc.tile_pool(name="psum", bufs=6, space="PSUM"))

        # bn2 params (shared)
        s2_sb = cpool.tile([128, CT, 1], F32)
        t2_sb = cpool.tile([128, CT, 1], F32)
        for co_t in range(CT):
            nc.scalar.dma_start(out=s2_sb[:, co_t], in_=s2_d[co_t * 128 : (co_t + 1) * 128])
            nc.scalar.dma_start(out=t2_sb[:, co_t], in_=t2_d[co_t * 128 : (co_t + 1) * 128])

        if not sparse:
            w1_sb = wpool.tile([128, CT, NS, c], mdt)
            w2_sb = wpool.tile([128, CT, NS, c], mdt)

        for i in range(bpc):
            nt = slot_tiles[i] if sparse else CT  # conv1 output tiles / conv2 K tiles
            np_i = 128 * nt

            # ---- masks (partition-broadcast DMA on the SWDGE ring) ----
            mask_pad = mpool.tile([128, FLAT], BF16, tag="m", name=f"mask{i}")
            m3 = mask_pad.rearrange("p (a b) -> p a b", b=PW)
            nc.vector.memset(m3[:, 1 : h + 1, 0:1], 0.0)
            nc.vector.memset(m3[:, 1 : h + 1, w + 1 : w + 2], 0.0)
            nc.gpsimd.dma_start(out=m3[:, 1 : h + 1, 1 : w + 1], in_=bcast(mask_d[i]))

            maskd_pad = mdpool.tile([128, FLAT], BF16, tag="md", name=f"maskd{i}")
            md3 = maskd_pad.rearrange("p (a b) -> p a b", b=PW)
            nc.vector.memset(md3[:, 0, :], 0.0)
            nc.vector.memset(md3[:, h + 1, :], 0.0)
            nc.vector.memset(md3[:, 1 : h + 1, 0:1], 0.0)
            nc.vector.memset(md3[:, 1 : h + 1, w + 1 : w + 2], 0.0)
            nc.gpsimd.dma_start(out=md3[:, 1 : h + 1, 1 : w + 1], in_=bcast(maskd_d[i]))

            # ---- x (padded, sync ring) and g = x * mask_dilate ----
            x_pad, g_pad = [], []
            for ci_t in range(CT):
                xt = xpool.tile([128, FLAT], F32, tag="x", name=f"x{i}_{ci_t}")
                x3 = xt.rearrange("p (a b) -> p a b", b=PW)
                nc.vector.memset(x3[:, 0, :], 0.0)
                nc.vector.memset(x3[:, h + 1, :], 0.0)
                nc.vector.memset(x3[:, 1 : h + 1, 0:1], 0.0)
                nc.vector.memset(x3[:, 1 : h + 1, w + 1 : w + 2], 0.0)
                nc.sync.dma_start(
                    out=x3[:, 1 : h + 1, 1 : w + 1],
                    in_=x_d[i, ci_t * 128 : (ci_t + 1) * 128].rearrange(
                        "p (a b) -> p a b", b=w
                    ),
                )
                x_pad.append(xt)
                gt = spool.tile([128, FLAT], mdt, tag="scr", name=f"g{i}_{ci_t}")
                nc.vector.tensor_mul(gt, xt, maskd_pad)
                g_pad.append(gt)

            # ---- weights for this image (scalar/HWDGE ring) ----
            if sparse:
                w1_sb = wpool.tile([128, CT, NS, np_i], mdt, tag="w1g", name=f"w1g{i}")
                for ci_t in range(CT):
                    nc.scalar.dma_start(out=w1_sb[:, ci_t], in_=w1_d[i][ci_t])
                w2_sb = wpool.tile([128, nt, NS, c], mdt, tag="w2g", name=f"w2g{i}")
                for ci_t in range(nt):
                    nc.scalar.dma_start(out=w2_sb[:, ci_t], in_=w2_d[i][ci_t])
            elif i == 0:
                for ci_t in range(CT):
                    nc.scalar.dma_start(out=w1_sb[:, ci_t], in_=w1s_d[ci_t])
                    nc.scalar.dma_start(out=w2_sb[:, ci_t], in_=w2s_d[ci_t])

            # ---- folded bn1*vector params ----
            s1v_t = ppool.tile([128, nt, 1], F32, tag="s1v", name=f"s1v{i}")
            t1v_t = ppool.tile([128, nt, 1], F32, tag="t1v", name=f"t1v{i}")
            for co_t in range(nt):
                if sparse:
                    nc.scalar.dma_start(
                        out=s1v_t[:, co_t], in_=s1_d[i][co_t * 128 : (co_t + 1) * 128]
                    )
                    nc.scalar.dma_start(
                        out=t1v_t[:, co_t], in_=t1_d[i][co_t * 128 : (co_t + 1) * 128]
                    )
                else:
                    nc.scalar.dma_start(
                        out=s1v_t[:, co_t], in_=s1v_d[i, co_t * 128 : (co_t + 1) * 128]
                    )
                    nc.scalar.dma_start(
                        out=t1v_t[:, co_t], in_=t1v_d[i, co_t * 128 : (co_t + 1) * 128]
                    )

            # ---- conv1 -> h (active channels only in sparse mode) ----
            h_pad = []
            for co_t in range(nt):
                ht = hpool.tile([128, FLAT], mdt, tag="h", name=f"h{i}_{co_t}")
                nc.vector.tensor_scalar_mul(ht[:, 0:out_lo], x_pad[0][:, 0:out_lo], 0.0)
                nc.vector.tensor_scalar_mul(
                    ht[:, out_hi + 1 : FLAT], x_pad[0][:, out_hi + 1 : FLAT], 0.0
                )
                h_pad.append(ht)

            for co_t in range(nt):
                for off, wd in chunks:
                    ps = pspool.tile([128, chunk_alloc], F32, tag="ps", name=f"ps1_{i}_{co_t}_{off}")
                    k, nk = 0, CT * NS
                    for ci_t in range(CT):
                        for s in range(NS):
                            nc.tensor.matmul(
                                ps[:, :wd],
                                w1_sb[:, ci_t, s, co_t * 128 : co_t * 128 + 128],
                                g_pad[ci_t][:, off + shifts[s] : off + shifts[s] + wd],
                                start=(k == 0),
                                stop=(k == nk - 1),
                            )
                            k += 1
                    r = epool.tile([128, chunk_alloc], edt, tag="e", name=f"r{i}_{co_t}_{off}")
                    nc.scalar.activation(
                        r[:, :wd], ps[:, :wd], Relu, bias=t1v_t[:, co_t], scale=s1v_t[:, co_t]
                    )
                    nc.vector.tensor_mul(
                        h_pad[co_t][:, off : off + wd], r[:, :wd], mask_pad[:, off : off + wd]
                    )

            # ---- conv2 -> out ----
            out_t = [
                spool.tile([128, FLAT], F32, tag="scr", name=f"o{i}_{ct}") for ct in range(CT)
            ]
            for co_t in range(CT):
                for off, wd in chunks:
                    ps = pspool.tile([128, chunk_alloc], F32, tag="ps", name=f"ps2_{i}_{co_t}_{off}")
                    k, nk = 0, nt * NS
                    for ci_t in range(nt):
                        for s in range(NS):
                            nc.tensor.matmul(
                                ps[:, :wd],
                                w2_sb[:, ci_t, s, co_t * 128 : co_t * 128 + 128],
                                h_pad[ci_t][:, off + shifts[s] : off + shifts[s] + wd],
                                start=(k == 0),
                                stop=(k == nk - 1),
                            )
                            k += 1
                    e = epool.tile([128, chunk_alloc], F32, tag="e", name=f"e{i}_{co_t}_{off}")
                    nc.scalar.activation(
                        e[:, :wd], ps[:, :wd], Ident, bias=t2_sb[:, co_t], scale=s2_sb[:, co_t]
                    )
                    nc.vector.tensor_mul(e[:, :wd], e[:, :wd], mask_pad[:, off : off + wd])
                    dst = out_t[co_t][:, off : off + wd]
                    nc.vector.tensor_add(dst, e[:, :wd], x_pad[co_t][:, off : off + wd])
                    nc.scalar.activation(dst, dst, Relu)

            for co_t in range(CT):
                o3 = out_t[co_t].rearrange("p (a b) -> p a b", b=PW)
                nc.sync.dma_start(
                    out=out_d[i, co_t * 128 : (co_t + 1) * 128].rearrange(
                        "p (a b) -> p a b", b=w
                    ),
                    in_=o3[:, 1 : h + 1, 1 : w + 1],
                )

    nc.compile()
    return nc


# ----------------------------------------------------------------------------
# host-side prep + execution
# ----------------------------------------------------------------------------
_NC_CACHE = {}


def _get_nc(key, **kw):
    if key not in _NC_CACHE:
        _NC_CACHE[key] = build_nc(**kw)
    return _NC_CACHE[key]


def _wt_np(mm_mode):
    return ml_dtypes.bfloat16 if mm_mode == "bf16" else np.float32


def _prep_weights(wt, mm_mode, c=C):
    # [co, ci, 3, 3] -> [ci_t, ci, s, co] with s = dy*3+dx
    t = np.ascontiguousarray(wt.transpose(1, 2, 3, 0).reshape(c // 128, 128, 9, c))
    return t.astype(_wt_np(mm_mode))


def kernel(**inputs):
    global LAST_EXEC_NS, LAST_TRACE
    x = np.asarray(inputs["x"], dtype=np.float32)
    mask = np.asarray(inputs["mask"], dtype=np.float32).reshape(B, H * W)
    maskd = np.asarray(inputs["mask_dilate"], dtype=np.float32).reshape(B, H * W)
    vector = np.asarray(inputs["vector"], dtype=np.float32)
    w1 = np.asarray(inputs["conv1_w"], dtype=np.float32)
    w2 = np.asarray(inputs["conv2_w"], dtype=np.float32)

    s1 = np.asarray(inputs["bn1_g"]) / np.sqrt(np.asarray(inputs["bn1_v"]) + EPS)
    t1 = np.asarray(inputs["bn1_b"]) - np.asarray(inputs["bn1_m"]) * s1
    s2 = np.asarray(inputs["bn2_g"]) / np.sqrt(np.asarray(inputs["bn2_v"]) + EPS)
    t2 = np.asarray(inputs["bn2_b"]) - np.asarray(inputs["bn2_m"]) * s2
    s1, t1 = s1.astype(np.float32), t1.astype(np.float32)

    binary = lambda a: bool(np.isin(a, (0.0, 1.0)).all())  # noqa: E731
    masks_binary = binary(mask) and binary(maskd)
    assert (vector >= 0).all() and masks_binary, (
        "kernel specialized for setup_inputs-style binary masks / nonneg vector"
    )
    use_sparse = SPARSE and binary(vector)

    mask_bf = mask.astype(ml_dtypes.bfloat16)
    maskd_bf = maskd.astype(ml_dtypes.bfloat16)
    mm_mode = MM_MODE
    wdt = _wt_np(mm_mode)

    if use_sparse:
        nact = vector.sum(1).astype(int)
        order = np.argsort(-nact, kind="stable")
        slots = order.reshape(BPC, NCORES)  # [slot, core] -> original image idx
        slot_tiles = tuple(
            max(1, int(np.ceil(nact[slots[s]].max() / 128))) for s in range(BPC)
        )
        if sum(slot_tiles) >= BPC * (C // 128):
            use_sparse = False  # no win; fall back to shared-weight dense kernel

    if use_sparse:
        nc = _get_nc(("sparse", mm_mode, slot_tiles), mm_mode=mm_mode, slot_tiles=slot_tiles)
        # full lhsT layouts to gather from
        w1l = w1.transpose(1, 2, 3, 0).reshape(C, 9, C)  # [ci, s, co]
        w2r = w2.transpose(1, 2, 3, 0).reshape(C, 9, C)  # [ci, s, co] rows = conv2 input ch
        in_maps = []
        for cid in range(NCORES):
            imgs = [int(slots[s, cid]) for s in range(BPC)]
            m = dict(
                x=np.ascontiguousarray(x.reshape(B, C, H * W)[imgs]),
                mask=np.ascontiguousarray(mask_bf[imgs]),
                maskd=np.ascontiguousarray(maskd_bf[imgs]),
                s2=np.ascontiguousarray(s2.reshape(C, 1).astype(np.float32)),
                t2=np.ascontiguousarray(t2.reshape(C, 1).astype(np.float32)),
            )
            for s, b in enumerate(imgs):
                np_s = 128 * slot_tiles[s]
                idx = np.where(vector[b] > 0)[0]
                k = len(idx)
                idxp = np.zeros(np_s, dtype=int)
                idxp[:k] = idx
                # conv1 weights gathered on OUTPUT channel; pad -> zero
                w1g = w1l[:, :, idxp].copy()  # [ci, s, np_s]
                w1g[:, :, k:] = 0
                m[f"w1g{s}"] = np.ascontiguousarray(
                    w1g.reshape(C // 128, 128, 9, np_s)
                ).astype(wdt)
                # conv2 weights gathered on INPUT channel; pad -> zero
                w2g = w2r[idxp].copy()  # [np_s, s, co]
                w2g[k:] = 0
                m[f"w2g{s}"] = np.ascontiguousarray(
                    w2g.reshape(slot_tiles[s], 128, 9, C)
                ).astype(wdt)
                sg = np.zeros(np_s, np.float32)
                tg = np.zeros(np_s, np.float32)
                sg[:k] = s1[idx]
                tg[:k] = t1[idx]
                m[f"s1vg{s}"] = sg.reshape(np_s, 1)
                m[f"t1vg{s}"] = tg.reshape(np_s, 1)
            in_maps.append(m)
    else:
        nc = _get_nc(("dense", mm_mode), mm_mode=mm_mode)
        s1v = (s1[None, :] * vector).astype(np.float32)
        t1v = (t1[None, :] * vector).astype(np.float32)
        w1l = _prep_weights(w1, mm_mode)
        w2l = _prep_weights(w2, mm_mode)
        xs = x.reshape(NCORES, BPC, C, H * W)
        in_maps = []
        for cid in range(NCORES):
            sl = slice(cid * BPC, (cid + 1) * BPC)
            in_maps.append(
                dict(
                    x=np.ascontiguousarray(xs[cid]),
                    mask=np.ascontiguousarray(mask_bf[sl]),
                    maskd=np.ascontiguousarray(maskd_bf[sl]),
                    w1=w1l,
                    w2=w2l,
                    s1v=np.ascontiguousarray(s1v[sl].reshape(BPC, C, 1)),
                    t1v=np.ascontiguousarray(t1v[sl].reshape(BPC, C, 1)),
                    s2=np.ascontiguousarray(s2.reshape(C, 1).astype(np.float32)),
                    t2=np.ascontiguousarray(t2.reshape(C, 1).astype(np.float32)),
                )
            )

    res = bass_utils.run_bass_kernel_spmd(
        nc, in_maps, core_ids=list(range(NCORES)), trace=TRACE
    )
    LAST_EXEC_NS = res.exec_time_ns
    LAST_TRACE = res.instructions_and_trace[1] if res.instructions_and_trace else None

    y = np.empty((B, C, H * W), np.float32)
    if use_sparse:
        for cid in range(NCORES):
            for s in range(BPC):
                y[int(slots[s, cid])] = res.results[cid]["out"][s]
    else:
        for cid in range(NCORES):
            y[cid * BPC : (cid + 1) * BPC] = res.results[cid]["out"]
    return y.reshape(B, C, H, W)


# revision 16
# speedup vs baseline: 1.3013x; 1.0836x over previous
"""Trainium2 Bass kernel for the sparse BasicBlock problem.

Math (masks and `vector` are binary in setup_inputs; verified at runtime):
    g   = x * mask_dilate
    c1  = conv3x3(g, w1)
    h   = relu(c1 * s1v + t1v) * mask      (BN1 affine folded with `vector`)
    c2  = conv3x3(h, w2)
    out = relu(x + (c2 * s2 + t2) * mask)

Layout: per image, channels on SBUF partitions, spatial flattened as a
zero-padded (H+2)x(W+2) row-major plane so a 3x3 conv is 9 shifted matmuls
accumulating in PSUM (shift = (dy-1)*(W+2) + (dx-1)). Matmuls run in fp32r
(full PE rate, ~1e-4 relative error).

Channel sparsity: `vector` zeroes ~half of conv1's output channels per image
(h == 0 there), so conv1 computes only the active channels (M-compaction) and
conv2 contracts only over them (K-compaction), via host-side gathered and
zero-padded per-image weights. One SPMD program is shared by all 8 cores, so
images are sorted by active-channel count and assigned so that each image
slot has a fixed channel-tile count across cores (max over cores).

Sharding: data-parallel over batch, 4 images per core on 8 cores.
"""

import sys
import types
from contextlib import ExitStack

sys.path.insert(0, "/opt/trn_rl_repo")

import ml_dtypes
import numpy as np

import concourse.bacc as bacc
import concourse.bass as bass
import concourse.mybir as mybir
import concourse.tile as tile
from concourse import bass_utils

# ----------------------------------------------------------------------------
# axon NTFF profiling hook shim (enables trace=True under axon)
# ----------------------------------------------------------------------------
_HOOK = {"hook": None}


def _install_axon_hooks():
    try:
        import antenv  # noqa: F401
    except ImportError:
        return
    if "antenv.axon_hooks" not in sys.modules:
        mod = types.ModuleType("antenv.axon_hooks")
        mod.set_axon_ntff_profile_hook = lambda h: _HOOK.__setitem__("hook", h)
        mod.get_axon_ntff_profile_hook = lambda: _HOOK["hook"]
        sys.modules["antenv.axon_hooks"] = mod
    if _HOOK["hook"] is None:
        try:
            from trn_agent_boot.trn_boot import _ntff_profile_via_ctypes

            sys.modules["antenv.axon_hooks"].set_axon_ntff_profile_hook(
                _ntff_profile_via_ctypes("/opt/axon/libaxon_pjrt.so")
            )
        except Exception:
            pass


_install_axon_hooks()
bass_utils.upload_artifacts = lambda tmpdir: tmpdir  # no S3 in this container

# ----------------------------------------------------------------------------
# problem constants (hardcoded per spec)
# ----------------------------------------------------------------------------
B, C, H, W = 32, 256, 56, 56
NCORES = 8
BPC = B // NCORES
EPS = 1e-5

TRACE = False
MM_MODE = "f32r"  # 'f32r' | 'bf16' | 'f32'
SPARSE = True
LAST_EXEC_NS = None
LAST_TRACE = None

F32 = mybir.dt.float32
BF16 = mybir.dt.bfloat16


def _chunks(total, maxw):
    """EVEN-width chunks <= maxw (fp32r needs an even moving dim; >=256 keeps
    full PE rate)."""
    assert total % 2 == 0, total
    n = -(-total // maxw)
    base = (total // n) & ~1
    rem = total - base * n
    out = []
    off = 0
    for i in range(n):
        w = base + (2 if i < rem // 2 else 0)
        out.append((off, w))
        off += w
    assert off == total
    return out


def _mdt(mm_mode):
    return {"f32r": mybir.dt.float32r, "bf16": BF16, "f32": F32}[mm_mode]


def build_nc(mm_mode=MM_MODE, bpc=BPC, c=C, h=H, w=W, slot_tiles=None):
    """Build the per-core SPMD Bass program.

    slot_tiles: None for the dense kernel, else per-image-slot channel-tile
    counts (e.g. (2, 2, 1, 1)) for the sparsity-specialized kernel.
    """
    PW, PH = w + 2, h + 2
    FLAT = PH * PW
    CT = c // 128
    NS = 9
    shifts = [(dy - 1) * PW + (dx - 1) for dy in range(3) for dx in range(3)]
    out_lo = PW + 1
    out_hi = h * PW + w
    span = out_hi - out_lo + 1
    chunks = [(out_lo + o, s) for (o, s) in _chunks(span, 464)]
    chunk_alloc = max(s for _, s in chunks)

    sparse = slot_tiles is not None
    if sparse:
        assert len(slot_tiles) == bpc
        max_nt = max(slot_tiles)
    mdt = _mdt(mm_mode)
    edt = F32 if mm_mode == "f32r" else mdt

    nc = bacc.Bacc("TRN2", debug=False, enable_asserts=False, num_devices=NCORES)

    # x / masks / out are passed HOST-PADDED to the (h+2)x(w+2) plane so every
    # large DMA is fully contiguous
    x_d = nc.dram_tensor("x", [bpc, c, FLAT], F32, kind="ExternalInput").ap()
    mask_d = nc.dram_tensor("mask", [bpc, FLAT], BF16, kind="ExternalInput").ap()
    maskd_d = nc.dram_tensor("maskd", [bpc, FLAT], BF16, kind="ExternalInput").ap()
    s2_d = nc.dram_tensor("s2", [c, 1], F32, kind="ExternalInput").ap()
    t2_d = nc.dram_tensor("t2", [c, 1], F32, kind="ExternalInput").ap()
    out_d = nc.dram_tensor("out", [bpc, c, FLAT], F32, kind="ExternalOutput").ap()
    if sparse:
        w1_d, w2_d, s1_d, t1_d = [], [], [], []
        for s, nt in enumerate(slot_tiles):
            np_s = 128 * nt
            w1_d.append(
                nc.dram_tensor(f"w1g{s}", [CT, 128, NS, np_s], mdt, kind="ExternalInput").ap()
            )
            w2_d.append(
                nc.dram_tensor(f"w2g{s}", [nt, 128, NS, c], mdt, kind="ExternalInput").ap()
            )
            s1_d.append(
                nc.dram_tensor(f"s1vg{s}", [np_s, 1], F32, kind="ExternalInput").ap()
            )
            t1_d.append(
                nc.dram_tensor(f"t1vg{s}", [np_s, 1], F32, kind="ExternalInput").ap()
            )
    else:
        w1s_d = nc.dram_tensor("w1", [CT, 128, NS, c], mdt, kind="ExternalInput").ap()
        w2s_d = nc.dram_tensor("w2", [CT, 128, NS, c], mdt, kind="ExternalInput").ap()
        s1v_d = nc.dram_tensor("s1v", [bpc, c, 1], F32, kind="ExternalInput").ap()
        t1v_d = nc.dram_tensor("t1v", [bpc, c, 1], F32, kind="ExternalInput").ap()

    Relu = mybir.ActivationFunctionType.Relu
    Ident = mybir.ActivationFunctionType.Identity

    with tile.TileContext(nc) as tc, ExitStack() as ctx:
        wpool = ctx.enter_context(tc.tile_pool(name="wpool", bufs=1))
        cpool = ctx.enter_context(tc.tile_pool(name="cpool", bufs=1))
        ppool = ctx.enter_context(tc.tile_pool(name="ppool", bufs=2))
        xpool = ctx.enter_context(tc.tile_pool(name="xpool", bufs=CT + 1))
        spool = ctx.enter_context(tc.tile_pool(name="spool", bufs=CT + 1))
        hpool = ctx.enter_context(
            tc.tile_pool(name="hpool", bufs=(max(2, max_nt) if sparse else CT))
        )
        mpool = ctx.enter_context(tc.tile_pool(name="mpool", bufs=2))
        mdpool = ctx.enter_context(tc.tile_pool(name="mdpool", bufs=2))
        epool = ctx.enter_context(tc.tile_pool(name="epool", bufs=6))
        pspool = ctx.enter_context(tc.tile_pool(name="psum", bufs=6, space="PSUM"))

        # bn2 params (shared)
        s2_sb = cpool.tile([128, CT, 1], F32)
        t2_sb = cpool.tile([128, CT, 1], F32)
        for co_t in range(CT):
            nc.scalar.dma_start(out=s2_sb[:, co_t], in_=s2_d[co_t * 128 : (co_t + 1) * 128])
            nc.scalar.dma_start(out=t2_sb[:, co_t], in_=t2_d[co_t * 128 : (co_t + 1) * 128])

        if not sparse:
            w1_sb = wpool.tile([128, CT, NS, c], mdt)
            w2_sb = wpool.tile([128, CT, NS, c], mdt)

        for i in range(bpc):
            nt = slot_tiles[i] if sparse else CT  # conv1 output tiles / conv2 K tiles
            np_i = 128 * nt

            # ---- masks: 1-row DMA into partition 0, then in-place broadcast ----
            maskd_pad = mdpool.tile([128, FLAT], BF16, tag="md", name=f"maskd{i}")
            nc.scalar.dma_start(out=maskd_pad[0:1, :], in_=maskd_d[i : i + 1])
            nc.gpsimd.partition_broadcast(maskd_pad, maskd_pad[0:1, :])

            mask_pad = mpool.tile([128, FLAT], BF16, tag="m", name=f"mask{i}")
            nc.scalar.dma_start(out=mask_pad[0:1, :], in_=mask_d[i : i + 1])
            nc.gpsimd.partition_broadcast(mask_pad, mask_pad[0:1, :])

            # ---- x (padded, sync ring) and g = x * mask_dilate ----
            x_pad, g_pad = [], []
            for ci_t in range(CT):
                xt = xpool.tile([128, FLAT], F32, tag="x", name=f"x{i}_{ci_t}")
                nc.sync.dma_start(out=xt, in_=x_d[i, ci_t * 128 : (ci_t + 1) * 128])
                x_pad.append(xt)
                gt = spool.tile([128, FLAT], mdt, tag="scr", name=f"g{i}_{ci_t}")
                nc.vector.tensor_mul(gt, xt, maskd_pad)
                g_pad.append(gt)

            # ---- weights for this image (scalar/HWDGE ring) ----
            if sparse:
                w1_sb = wpool.tile([128, CT, NS, np_i], mdt, tag="w1g", name=f"w1g{i}")
                for ci_t in range(CT):
                    nc.scalar.dma_start(out=w1_sb[:, ci_t], in_=w1_d[i][ci_t])
                w2_sb = wpool.tile([128, nt, NS, c], mdt, tag="w2g", name=f"w2g{i}")
                for ci_t in range(nt):
                    nc.scalar.dma_start(out=w2_sb[:, ci_t], in_=w2_d[i][ci_t])
            elif i == 0:
                for ci_t in range(CT):
                    nc.scalar.dma_start(out=w1_sb[:, ci_t], in_=w1s_d[ci_t])
                    nc.scalar.dma_start(out=w2_sb[:, ci_t], in_=w2s_d[ci_t])

            # ---- folded bn1*vector params ----
            s1v_t = ppool.tile([128, nt, 1], F32, tag="s1v", name=f"s1v{i}")
            t1v_t = ppool.tile([128, nt, 1], F32, tag="t1v", name=f"t1v{i}")
            for co_t in range(nt):
                if sparse:
                    nc.scalar.dma_start(
                        out=s1v_t[:, co_t], in_=s1_d[i][co_t * 128 : (co_t + 1) * 128]
                    )
                    nc.scalar.dma_start(
                        out=t1v_t[:, co_t], in_=t1_d[i][co_t * 128 : (co_t + 1) * 128]
                    )
                else:
                    nc.scalar.dma_start(
                        out=s1v_t[:, co_t], in_=s1v_d[i, co_t * 128 : (co_t + 1) * 128]
                    )
                    nc.scalar.dma_start(
                        out=t1v_t[:, co_t], in_=t1v_d[i, co_t * 128 : (co_t + 1) * 128]
                    )

            # ---- conv1 -> h (active channels only in sparse mode) ----
            h_pad = []
            for co_t in range(nt):
                ht = hpool.tile([128, FLAT], mdt, tag="h", name=f"h{i}_{co_t}")
                nc.vector.tensor_scalar_mul(ht[:, 0:out_lo], x_pad[0][:, 0:out_lo], 0.0)
                nc.vector.tensor_scalar_mul(
                    ht[:, out_hi + 1 : FLAT], x_pad[0][:, out_hi + 1 : FLAT], 0.0
                )
                h_pad.append(ht)

            for co_t in range(nt):
                for off, wd in chunks:
                    ps = pspool.tile([128, chunk_alloc], F32, tag="ps", name=f"ps1_{i}_{co_t}_{off}")
                    k, nk = 0, CT * NS
                    for ci_t in range(CT):
                        for s in range(NS):
                            nc.tensor.matmul(
                                ps[:, :wd],
                                w1_sb[:, ci_t, s, co_t * 128 : co_t * 128 + 128],
                                g_pad[ci_t][:, off + shifts[s] : off + shifts[s] + wd],
                                start=(k == 0),
                                stop=(k == nk - 1),
                            )
                            k += 1
                    r = epool.tile([128, chunk_alloc], edt, tag="e", name=f"r{i}_{co_t}_{off}")
                    nc.scalar.activation(
                        r[:, :wd], ps[:, :wd], Relu, bias=t1v_t[:, co_t], scale=s1v_t[:, co_t]
                    )
                    nc.vector.tensor_mul(
                        h_pad[co_t][:, off : off + wd], r[:, :wd], mask_pad[:, off : off + wd]
                    )

            # ---- conv2 -> out ----
            out_t = []
            for ct in range(CT):
                ot = spool.tile([128, FLAT], F32, tag="scr", name=f"o{i}_{ct}")
                nc.vector.memset(ot[:, 0:out_lo], 0.0)
                nc.vector.memset(ot[:, out_hi + 1 : FLAT], 0.0)
                out_t.append(ot)
            for co_t in range(CT):
                for off, wd in chunks:
                    ps = pspool.tile([128, chunk_alloc], F32, tag="ps", name=f"ps2_{i}_{co_t}_{off}")
                    k, nk = 0, nt * NS
                    for ci_t in range(nt):
                        for s in range(NS):
                            nc.tensor.matmul(
                                ps[:, :wd],
                                w2_sb[:, ci_t, s, co_t * 128 : co_t * 128 + 128],
                                h_pad[ci_t][:, off + shifts[s] : off + shifts[s] + wd],
                                start=(k == 0),
                                stop=(k == nk - 1),
                            )
                            k += 1
                    e = epool.tile([128, chunk_alloc], F32, tag="e", name=f"e{i}_{co_t}_{off}")
                    nc.scalar.activation(
                        e[:, :wd], ps[:, :wd], Ident, bias=t2_sb[:, co_t], scale=s2_sb[:, co_t]
                    )
                    nc.vector.tensor_mul(e[:, :wd], e[:, :wd], mask_pad[:, off : off + wd])
                    dst = out_t[co_t][:, off : off + wd]
                    nc.vector.tensor_add(dst, e[:, :wd], x_pad[co_t][:, off : off + wd])
                    nc.scalar.activation(dst, dst, Relu)

            for co_t in range(CT):
                nc.sync.dma_start(
                    out=out_d[i, co_t * 128 : (co_t + 1) * 128], in_=out_t[co_t]
                )

    nc.compile()
    return nc


# ----------------------------------------------------------------------------
# host-side prep + execution
# ----------------------------------------------------------------------------
_NC_CACHE = {}


def _get_nc(key, **kw):
    if key not in _NC_CACHE:
        _NC_CACHE[key] = build_nc(**kw)
    return _NC_CACHE[key]


def _wt_np(mm_mode):
    return ml_dtypes.bfloat16 if mm_mode == "bf16" else np.float32


def _prep_weights(wt, mm_mode, c=C):
    # [co, ci, 3, 3] -> [ci_t, ci, s, co] with s = dy*3+dx
    t = np.ascontiguousarray(wt.transpose(1, 2, 3, 0).reshape(c // 128, 128, 9, c))
    return t.astype(_wt_np(mm_mode))


def kernel(**inputs):
    global LAST_EXEC_NS, LAST_TRACE
    x = np.asarray(inputs["x"], dtype=np.float32)
    mask = np.asarray(inputs["mask"], dtype=np.float32).reshape(B, H * W)
    maskd = np.asarray(inputs["mask_dilate"], dtype=np.float32).reshape(B, H * W)
    vector = np.asarray(inputs["vector"], dtype=np.float32)
    w1 = np.asarray(inputs["conv1_w"], dtype=np.float32)
    w2 = np.asarray(inputs["conv2_w"], dtype=np.float32)

    s1 = np.asarray(inputs["bn1_g"]) / np.sqrt(np.asarray(inputs["bn1_v"]) + EPS)
    t1 = np.asarray(inputs["bn1_b"]) - np.asarray(inputs["bn1_m"]) * s1
    s2 = np.asarray(inputs["bn2_g"]) / np.sqrt(np.asarray(inputs["bn2_v"]) + EPS)
    t2 = np.asarray(inputs["bn2_b"]) - np.asarray(inputs["bn2_m"]) * s2
    s1, t1 = s1.astype(np.float32), t1.astype(np.float32)

    binary = lambda a: bool(np.isin(a, (0.0, 1.0)).all())  # noqa: E731
    masks_binary = binary(mask) and binary(maskd)
    assert (vector >= 0).all() and masks_binary, (
        "kernel specialized for setup_inputs-style binary masks / nonneg vector"
    )
    use_sparse = SPARSE and binary(vector)

    # host-pad x and masks to the (H+2)x(W+2) plane => contiguous device DMAs
    PW, PH = W + 2, H + 2
    FLAT = PH * PW
    xp = np.zeros((B, C, PH, PW), np.float32)
    xp[:, :, 1 : H + 1, 1 : W + 1] = x
    xp = xp.reshape(B, C, FLAT)
    mask_bf = np.zeros((B, PH, PW), ml_dtypes.bfloat16)
    mask_bf[:, 1 : H + 1, 1 : W + 1] = mask.reshape(B, H, W)
    mask_bf = mask_bf.reshape(B, FLAT)
    maskd_bf = np.zeros((B, PH, PW), ml_dtypes.bfloat16)
    maskd_bf[:, 1 : H + 1, 1 : W + 1] = maskd.reshape(B, H, W)
    maskd_bf = maskd_bf.reshape(B, FLAT)
    mm_mode = MM_MODE
    wdt = _wt_np(mm_mode)

    if use_sparse:
        nact = vector.sum(1).astype(int)
        order = np.argsort(-nact, kind="stable")
        slots = order.reshape(BPC, NCORES)  # [slot, core] -> original image idx
        slot_tiles = tuple(
            max(1, int(np.ceil(nact[slots[s]].max() / 128))) for s in range(BPC)
        )
        if sum(slot_tiles) >= BPC * (C // 128):
            use_sparse = False  # no win; fall back to shared-weight dense kernel

    if use_sparse:
        nc = _get_nc(("sparse", mm_mode, slot_tiles), mm_mode=mm_mode, slot_tiles=slot_tiles)
        # full lhsT layouts to gather from
        w1l = w1.transpose(1, 2, 3, 0).reshape(C, 9, C)  # [ci, s, co]
        w2r = w2.transpose(1, 2, 3, 0).reshape(C, 9, C)  # [ci, s, co] rows = conv2 input ch
        in_maps = []
        for cid in range(NCORES):
            imgs = [int(slots[s, cid]) for s in range(BPC)]
            m = dict(
                x=np.ascontiguousarray(xp[imgs]),
                mask=np.ascontiguousarray(mask_bf[imgs]),
                maskd=np.ascontiguousarray(maskd_bf[imgs]),
                s2=np.ascontiguousarray(s2.reshape(C, 1).astype(np.float32)),
                t2=np.ascontiguousarray(t2.reshape(C, 1).astype(np.float32)),
            )
            for s, b in enumerate(imgs):
                np_s = 128 * slot_tiles[s]
                idx = np.where(vector[b] > 0)[0]
                k = len(idx)
                idxp = np.zeros(np_s, dtype=int)
                idxp[:k] = idx
                # conv1 weights gathered on OUTPUT channel; pad -> zero
                w1g = w1l[:, :, idxp].copy()  # [ci, s, np_s]
                w1g[:, :, k:] = 0
                m[f"w1g{s}"] = np.ascontiguousarray(
                    w1g.reshape(C // 128, 128, 9, np_s)
                ).astype(wdt)
                # conv2 weights gathered on INPUT channel; pad -> zero
                w2g = w2r[idxp].copy()  # [np_s, s, co]
                w2g[k:] = 0
                m[f"w2g{s}"] = np.ascontiguousarray(
                    w2g.reshape(slot_tiles[s], 128, 9, C)
                ).astype(wdt)
                sg = np.zeros(np_s, np.float32)
                tg = np.zeros(np_s, np.float32)
                sg[:k] = s1[idx]
                tg[:k] = t1[idx]
                m[f"s1vg{s}"] = sg.reshape(np_s, 1)
                m[f"t1vg{s}"] = tg.reshape(np_s, 1)
            in_maps.append(m)
    else:
        nc = _get_nc(("dense", mm_mode), mm_mode=mm_mode)
        s1v = (s1[None, :] * vector).astype(np.float32)
        t1v = (t1[None, :] * vector).astype(np.float32)
        w1l = _prep_weights(w1, mm_mode)
        w2l = _prep_weights(w2, mm_mode)
        xs = xp.reshape(NCORES, BPC, C, FLAT)
        in_maps = []
        for cid in range(NCORES):
            sl = slice(cid * BPC, (cid + 1) * BPC)
            in_maps.append(
                dict(
                    x=np.ascontiguousarray(xs[cid]),
                    mask=np.ascontiguousarray(mask_bf[sl]),
                    maskd=np.ascontiguousarray(maskd_bf[sl]),
                    w1=w1l,
                    w2=w2l,
                    s1v=np.ascontiguousarray(s1v[sl].reshape(BPC, C, 1)),
                    t1v=np.ascontiguousarray(t1v[sl].reshape(BPC, C, 1)),
                    s2=np.ascontiguousarray(s2.reshape(C, 1).astype(np.float32)),
                    t2=np.ascontiguousarray(t2.reshape(C, 1).astype(np.float32)),
                )
            )

    res = bass_utils.run_bass_kernel_spmd(
        nc, in_maps, core_ids=list(range(NCORES)), trace=TRACE
    )
    LAST_EXEC_NS = res.exec_time_ns
    LAST_TRACE = res.instructions_and_trace[1] if res.instructions_and_trace else None

    y = np.empty((B, C, FLAT), np.float32)
    if use_sparse:
        for cid in range(NCORES):
            for s in range(BPC):
                y[int(slots[s, cid])] = res.results[cid]["out"][s]
    else:
        for cid in range(NCORES):
            y[cid * BPC : (cid + 1) * BPC] = res.results[cid]["out"]
    return np.ascontiguousarray(
        y.reshape(B, C, PH, PW)[:, :, 1 : H + 1, 1 : W + 1]
    )


# revision 18
# speedup vs baseline: 1.3062x; 1.0038x over previous
"""Trainium2 Bass kernel for the sparse BasicBlock problem.

Math (masks and `vector` are binary in setup_inputs; verified at runtime):
    g   = x * mask_dilate
    c1  = conv3x3(g, w1)
    h   = relu(c1 * s1v + t1v) * mask      (BN1 affine folded with `vector`)
    c2  = conv3x3(h, w2)
    out = relu(x + (c2 * s2 + t2) * mask)

Layout: per image, channels on SBUF partitions, spatial flattened as a
zero-padded (H+2)x(W+2) row-major plane so a 3x3 conv is 9 shifted matmuls
accumulating in PSUM (shift = (dy-1)*(W+2) + (dx-1)). Matmuls run in fp32r
(full PE rate, ~1e-4 relative error).

Channel sparsity: `vector` zeroes ~half of conv1's output channels per image
(h == 0 there), so conv1 computes only the active channels (M-compaction) and
conv2 contracts only over them (K-compaction), via host-side gathered and
zero-padded per-image weights. One SPMD program is shared by all 8 cores, so
images are sorted by active-channel count and assigned so that each image
slot has a fixed channel-tile count across cores (max over cores).

Sharding: data-parallel over batch, 4 images per core on 8 cores.
"""

import sys
import types
from contextlib import ExitStack

sys.path.insert(0, "/opt/trn_rl_repo")

import ml_dtypes
import numpy as np

import concourse.bacc as bacc
import concourse.bass as bass
import concourse.mybir as mybir
import concourse.tile as tile
from concourse import bass_utils

# ----------------------------------------------------------------------------
# axon NTFF profiling hook shim (enables trace=True under axon)
# ----------------------------------------------------------------------------
_HOOK = {"hook": None}


def _install_axon_hooks():
    try:
        import antenv  # noqa: F401
    except ImportError:
        return
    if "antenv.axon_hooks" not in sys.modules:
        mod = types.ModuleType("antenv.axon_hooks")
        mod.set_axon_ntff_profile_hook = lambda h: _HOOK.__setitem__("hook", h)
        mod.get_axon_ntff_profile_hook = lambda: _HOOK["hook"]
        sys.modules["antenv.axon_hooks"] = mod
    if _HOOK["hook"] is None:
        try:
            from trn_agent_boot.trn_boot import _ntff_profile_via_ctypes

            sys.modules["antenv.axon_hooks"].set_axon_ntff_profile_hook(
                _ntff_profile_via_ctypes("/opt/axon/libaxon_pjrt.so")
            )
        except Exception:
            pass


_install_axon_hooks()
bass_utils.upload_artifacts = lambda tmpdir: tmpdir  # no S3 in this container

# ----------------------------------------------------------------------------
# problem constants (hardcoded per spec)
# ----------------------------------------------------------------------------
B, C, H, W = 32, 256, 56, 56
NCORES = 8
BPC = B // NCORES
EPS = 1e-5

TRACE = False
MM_MODE = "f32r"  # 'f32r' | 'bf16' | 'f32'
SPARSE = True
LAST_EXEC_NS = None
LAST_TRACE = None

F32 = mybir.dt.float32
BF16 = mybir.dt.bfloat16


def _chunks(total, maxw):
    """EVEN-width chunks <= maxw (fp32r needs an even moving dim; >=256 keeps
    full PE rate)."""
    assert total % 2 == 0, total
    n = -(-total // maxw)
    base = (total // n) & ~1
    rem = total - base * n
    out = []
    off = 0
    for i in range(n):
        w = base + (2 if i < rem // 2 else 0)
        out.append((off, w))
        off += w
    assert off == total
    return out


def _mdt(mm_mode):
    return {"f32r": mybir.dt.float32r, "bf16": BF16, "f32": F32}[mm_mode]


def build_nc(mm_mode=MM_MODE, bpc=BPC, c=C, h=H, w=W, slot_tiles=None):
    """Build the per-core SPMD Bass program.

    slot_tiles: None for the dense kernel, else per-image-slot channel-tile
    counts (e.g. (2, 2, 1, 1)) for the sparsity-specialized kernel.
    """
    PW, PH = w + 2, h + 2
    FLAT = PH * PW
    CT = c // 128
    NS = 9
    shifts = [(dy - 1) * PW + (dx - 1) for dy in range(3) for dx in range(3)]
    out_lo = PW + 1
    out_hi = h * PW + w
    span = out_hi - out_lo + 1
    chunks = [(out_lo + o, s) for (o, s) in _chunks(span, 464)]
    chunk_alloc = max(s for _, s in chunks)

    sparse = slot_tiles is not None
    if sparse:
        assert len(slot_tiles) == bpc
        max_nt = max(slot_tiles)
    mdt = _mdt(mm_mode)
    edt = F32 if mm_mode == "f32r" else mdt

    nc = bacc.Bacc("TRN2", debug=False, enable_asserts=False, num_devices=NCORES)

    # x / masks / out are passed HOST-PADDED to the (h+2)x(w+2) plane so every
    # large DMA is fully contiguous
    x_d = nc.dram_tensor("x", [bpc, c, FLAT], F32, kind="ExternalInput").ap()
    mask_d = nc.dram_tensor("mask", [bpc, FLAT], BF16, kind="ExternalInput").ap()
    maskd_d = nc.dram_tensor("maskd", [bpc, FLAT], BF16, kind="ExternalInput").ap()
    s2_d = nc.dram_tensor("s2", [c, 1], F32, kind="ExternalInput").ap()
    t2_d = nc.dram_tensor("t2", [c, 1], F32, kind="ExternalInput").ap()
    out_d = nc.dram_tensor("out", [bpc, c, FLAT], F32, kind="ExternalOutput").ap()
    if sparse:
        w1_d, w2_d, s1_d, t1_d = [], [], [], []
        for s, nt in enumerate(slot_tiles):
            np_s = 128 * nt
            w1_d.append(
                nc.dram_tensor(f"w1g{s}", [CT, 128, NS, np_s], mdt, kind="ExternalInput").ap()
            )
            w2_d.append(
                nc.dram_tensor(f"w2g{s}", [nt, 128, NS, c], mdt, kind="ExternalInput").ap()
            )
            s1_d.append(
                nc.dram_tensor(f"s1vg{s}", [np_s, 1], F32, kind="ExternalInput").ap()
            )
            t1_d.append(
                nc.dram_tensor(f"t1vg{s}", [np_s, 1], F32, kind="ExternalInput").ap()
            )
    else:
        w1s_d = nc.dram_tensor("w1", [CT, 128, NS, c], mdt, kind="ExternalInput").ap()
        w2s_d = nc.dram_tensor("w2", [CT, 128, NS, c], mdt, kind="ExternalInput").ap()
        s1v_d = nc.dram_tensor("s1v", [bpc, c, 1], F32, kind="ExternalInput").ap()
        t1v_d = nc.dram_tensor("t1v", [bpc, c, 1], F32, kind="ExternalInput").ap()

    Relu = mybir.ActivationFunctionType.Relu
    Ident = mybir.ActivationFunctionType.Identity

    with tile.TileContext(nc) as tc, ExitStack() as ctx:
        wpool = ctx.enter_context(tc.tile_pool(name="wpool", bufs=1))
        w1pool = ctx.enter_context(tc.tile_pool(name="w1pool", bufs=2))
        cpool = ctx.enter_context(tc.tile_pool(name="cpool", bufs=1))
        ppool = ctx.enter_context(tc.tile_pool(name="ppool", bufs=2))
        xpool = ctx.enter_context(tc.tile_pool(name="xpool", bufs=CT + 1))
        spool = ctx.enter_context(tc.tile_pool(name="spool", bufs=CT + 1))
        hpool = ctx.enter_context(
            tc.tile_pool(name="hpool", bufs=(max(2, max_nt) if sparse else CT))
        )
        mpool = ctx.enter_context(tc.tile_pool(name="mpool", bufs=2))
        mdpool = ctx.enter_context(tc.tile_pool(name="mdpool", bufs=2))
        epool = ctx.enter_context(tc.tile_pool(name="epool", bufs=8))
        pspool = ctx.enter_context(tc.tile_pool(name="psum", bufs=8, space="PSUM"))

        # bn2 params (shared)
        s2_sb = cpool.tile([128, CT, 1], F32)
        t2_sb = cpool.tile([128, CT, 1], F32)
        for co_t in range(CT):
            nc.scalar.dma_start(out=s2_sb[:, co_t], in_=s2_d[co_t * 128 : (co_t + 1) * 128])
            nc.scalar.dma_start(out=t2_sb[:, co_t], in_=t2_d[co_t * 128 : (co_t + 1) * 128])

        if not sparse:
            w1_sb = wpool.tile([128, CT, NS, c], mdt)
            w2_sb = wpool.tile([128, CT, NS, c], mdt)

        for i in range(bpc):
            nt = slot_tiles[i] if sparse else CT  # conv1 output tiles / conv2 K tiles
            np_i = 128 * nt

            # ---- masks: 1-row DMA into partition 0, then in-place broadcast ----
            maskd_pad = mdpool.tile([128, FLAT], BF16, tag="md", name=f"maskd{i}")
            nc.sync.dma_start(out=maskd_pad[0:1, :], in_=maskd_d[i : i + 1])
            nc.gpsimd.partition_broadcast(maskd_pad, maskd_pad[0:1, :])

            mask_pad = mpool.tile([128, FLAT], BF16, tag="m", name=f"mask{i}")
            nc.sync.dma_start(out=mask_pad[0:1, :], in_=mask_d[i : i + 1])

            # ---- x (padded, sync ring) and g = x * mask_dilate ----
            x_pad, g_pad = [], []
            for ci_t in range(CT):
                xt = xpool.tile([128, FLAT], F32, tag="x", name=f"x{i}_{ci_t}")
                eng = nc.sync if ci_t == 0 else nc.gpsimd
                eng.dma_start(out=xt, in_=x_d[i, ci_t * 128 : (ci_t + 1) * 128])
                x_pad.append(xt)
                gt = spool.tile([128, FLAT], mdt, tag="scr", name=f"g{i}_{ci_t}")
                nc.vector.tensor_mul(gt, xt, maskd_pad)
                g_pad.append(gt)
            nc.gpsimd.partition_broadcast(mask_pad, mask_pad[0:1, :])

            # ---- weights for this image (scalar/HWDGE ring) ----
            if sparse:
                w1_sb = w1pool.tile([128, CT, NS, np_i], mdt, tag="w1g", name=f"w1g{i}")
                for ci_t in range(CT):
                    nc.scalar.dma_start(out=w1_sb[:, ci_t], in_=w1_d[i][ci_t])
                w2_sb = wpool.tile([128, nt, NS, c], mdt, tag="w2g", name=f"w2g{i}")
                for ci_t in range(nt):
                    nc.scalar.dma_start(out=w2_sb[:, ci_t], in_=w2_d[i][ci_t])
            elif i == 0:
                for ci_t in range(CT):
                    nc.scalar.dma_start(out=w1_sb[:, ci_t], in_=w1s_d[ci_t])
                    nc.scalar.dma_start(out=w2_sb[:, ci_t], in_=w2s_d[ci_t])

            # ---- folded bn1*vector params ----
            s1v_t = ppool.tile([128, nt, 1], F32, tag="s1v", name=f"s1v{i}")
            t1v_t = ppool.tile([128, nt, 1], F32, tag="t1v", name=f"t1v{i}")
            for co_t in range(nt):
                if sparse:
                    nc.scalar.dma_start(
                        out=s1v_t[:, co_t], in_=s1_d[i][co_t * 128 : (co_t + 1) * 128]
                    )
                    nc.scalar.dma_start(
                        out=t1v_t[:, co_t], in_=t1_d[i][co_t * 128 : (co_t + 1) * 128]
                    )
                else:
                    nc.scalar.dma_start(
                        out=s1v_t[:, co_t], in_=s1v_d[i, co_t * 128 : (co_t + 1) * 128]
                    )
                    nc.scalar.dma_start(
                        out=t1v_t[:, co_t], in_=t1v_d[i, co_t * 128 : (co_t + 1) * 128]
                    )

            # ---- conv1 -> h (active channels only in sparse mode) ----
            h_pad = []
            for co_t in range(nt):
                ht = hpool.tile([128, FLAT], mdt, tag="h", name=f"h{i}_{co_t}")
                nc.vector.tensor_scalar_mul(ht[:, 0:out_lo], x_pad[0][:, 0:out_lo], 0.0)
                nc.vector.tensor_scalar_mul(
                    ht[:, out_hi + 1 : FLAT], x_pad[0][:, out_hi + 1 : FLAT], 0.0
                )
                h_pad.append(ht)

            for co_t in range(nt):
                for off, wd in chunks:
                    ps = pspool.tile([128, chunk_alloc], F32, tag="ps", name=f"ps1_{i}_{co_t}_{off}")
                    k, nk = 0, CT * NS
                    for ci_t in range(CT):
                        for s in range(NS):
                            nc.tensor.matmul(
                                ps[:, :wd],
                                w1_sb[:, ci_t, s, co_t * 128 : co_t * 128 + 128],
                                g_pad[ci_t][:, off + shifts[s] : off + shifts[s] + wd],
                                start=(k == 0),
                                stop=(k == nk - 1),
                            )
                            k += 1
                    r = epool.tile([128, chunk_alloc], edt, tag="e", name=f"r{i}_{co_t}_{off}")
                    nc.scalar.activation(
                        r[:, :wd], ps[:, :wd], Relu, bias=t1v_t[:, co_t], scale=s1v_t[:, co_t]
                    )
                    nc.vector.tensor_mul(
                        h_pad[co_t][:, off : off + wd], r[:, :wd], mask_pad[:, off : off + wd]
                    )

            # ---- conv2 -> out ----
            out_t = []
            for ct in range(CT):
                ot = spool.tile([128, FLAT], F32, tag="scr", name=f"o{i}_{ct}")
                nc.vector.memset(ot[:, 0:out_lo], 0.0)
                nc.vector.memset(ot[:, out_hi + 1 : FLAT], 0.0)
                out_t.append(ot)
            for co_t in range(CT):
                for off, wd in chunks:
                    ps = pspool.tile([128, chunk_alloc], F32, tag="ps", name=f"ps2_{i}_{co_t}_{off}")
                    k, nk = 0, nt * NS
                    for ci_t in range(nt):
                        for s in range(NS):
                            nc.tensor.matmul(
                                ps[:, :wd],
                                w2_sb[:, ci_t, s, co_t * 128 : co_t * 128 + 128],
                                h_pad[ci_t][:, off + shifts[s] : off + shifts[s] + wd],
                                start=(k == 0),
                                stop=(k == nk - 1),
                            )
                            k += 1
                    e = epool.tile([128, chunk_alloc], F32, tag="e", name=f"e{i}_{co_t}_{off}")
                    nc.scalar.activation(
                        e[:, :wd], ps[:, :wd], Ident, bias=t2_sb[:, co_t], scale=s2_sb[:, co_t]
                    )
                    nc.vector.tensor_mul(e[:, :wd], e[:, :wd], mask_pad[:, off : off + wd])
                    dst = out_t[co_t][:, off : off + wd]
                    nc.vector.tensor_add(dst, e[:, :wd], x_pad[co_t][:, off : off + wd])
                    nc.scalar.activation(dst, dst, Relu)

            for co_t in range(CT):
                eng = nc.sync if co_t == 0 else nc.scalar
                eng.dma_start(
                    out=out_d[i, co_t * 128 : (co_t + 1) * 128], in_=out_t[co_t]
                )

    nc.compile()
    return nc


# ----------------------------------------------------------------------------
# host-side prep + execution
# ----------------------------------------------------------------------------
_NC_CACHE = {}


def _get_nc(key, **kw):
    if key not in _NC_CACHE:
        _NC_CACHE[key] = build_nc(**kw)
    return _NC_CACHE[key]


def _wt_np(mm_mode):
    return ml_dtypes.bfloat16 if mm_mode == "bf16" else np.float32


def _prep_weights(wt, mm_mode, c=C):
    # [co, ci, 3, 3] -> [ci_t, ci, s, co] with s = dy*3+dx
    t = np.ascontiguousarray(wt.transpose(1, 2, 3, 0).reshape(c // 128, 128, 9, c))
    return t.astype(_wt_np(mm_mode))


def kernel(**inputs):
    global LAST_EXEC_NS, LAST_TRACE
    x = np.asarray(inputs["x"], dtype=np.float32)
    mask = np.asarray(inputs["mask"], dtype=np.float32).reshape(B, H * W)
    maskd = np.asarray(inputs["mask_dilate"], dtype=np.float32).reshape(B, H * W)
    vector = np.asarray(inputs["vector"], dtype=np.float32)
    w1 = np.asarray(inputs["conv1_w"], dtype=np.float32)
    w2 = np.asarray(inputs["conv2_w"], dtype=np.float32)

    s1 = np.asarray(inputs["bn1_g"]) / np.sqrt(np.asarray(inputs["bn1_v"]) + EPS)
    t1 = np.asarray(inputs["bn1_b"]) - np.asarray(inputs["bn1_m"]) * s1
    s2 = np.asarray(inputs["bn2_g"]) / np.sqrt(np.asarray(inputs["bn2_v"]) + EPS)
    t2 = np.asarray(inputs["bn2_b"]) - np.asarray(inputs["bn2_m"]) * s2
    s1, t1 = s1.astype(np.float32), t1.astype(np.float32)

    binary = lambda a: bool(np.isin(a, (0.0, 1.0)).all())  # noqa: E731
    masks_binary = binary(mask) and binary(maskd)
    assert (vector >= 0).all() and masks_binary, (
        "kernel specialized for setup_inputs-style binary masks / nonneg vector"
    )
    use_sparse = SPARSE and binary(vector)

    # host-pad x and masks to the (H+2)x(W+2) plane => contiguous device DMAs
    PW, PH = W + 2, H + 2
    FLAT = PH * PW
    xp = np.zeros((B, C, PH, PW), np.float32)
    xp[:, :, 1 : H + 1, 1 : W + 1] = x
    xp = xp.reshape(B, C, FLAT)
    mask_bf = np.zeros((B, PH, PW), ml_dtypes.bfloat16)
    mask_bf[:, 1 : H + 1, 1 : W + 1] = mask.reshape(B, H, W)
    mask_bf = mask_bf.reshape(B, FLAT)
    maskd_bf = np.zeros((B, PH, PW), ml_dtypes.bfloat16)
    maskd_bf[:, 1 : H + 1, 1 : W + 1] = maskd.reshape(B, H, W)
    maskd_bf = maskd_bf.reshape(B, FLAT)
    mm_mode = MM_MODE
    wdt = _wt_np(mm_mode)

    if use_sparse:
        nact = vector.sum(1).astype(int)
        order = np.argsort(-nact, kind="stable")
        slots = order.reshape(BPC, NCORES)  # [slot, core] -> original image idx
        # put a cheap (low tile-count) slot first so image 0's setup is light,
        # then the heavy slots
        rank = np.argsort([nact[slots[s]].max() for s in range(BPC)])
        perm = []
        light, heavy = list(rank), []
        if BPC >= 2:
            light, heavy = [rank[0]], list(rank[1:][::-1])
        perm = light + heavy
        slots = slots[perm]
        slot_tiles = tuple(
            max(1, int(np.ceil(nact[slots[s]].max() / 128))) for s in range(BPC)
        )
        if sum(slot_tiles) >= BPC * (C // 128):
            use_sparse = False  # no win; fall back to shared-weight dense kernel

    if use_sparse:
        nc = _get_nc(("sparse", mm_mode, slot_tiles), mm_mode=mm_mode, slot_tiles=slot_tiles)
        # full lhsT layouts to gather from
        w1l = w1.transpose(1, 2, 3, 0).reshape(C, 9, C)  # [ci, s, co]
        w2r = w2.transpose(1, 2, 3, 0).reshape(C, 9, C)  # [ci, s, co] rows = conv2 input ch
        in_maps = []
        for cid in range(NCORES):
            imgs = [int(slots[s, cid]) for s in range(BPC)]
            m = dict(
                x=np.ascontiguousarray(xp[imgs]),
                mask=np.ascontiguousarray(mask_bf[imgs]),
                maskd=np.ascontiguousarray(maskd_bf[imgs]),
                s2=np.ascontiguousarray(s2.reshape(C, 1).astype(np.float32)),
                t2=np.ascontiguousarray(t2.reshape(C, 1).astype(np.float32)),
            )
            for s, b in enumerate(imgs):
                np_s = 128 * slot_tiles[s]
                idx = np.where(vector[b] > 0)[0]
                k = len(idx)
                idxp = np.zeros(np_s, dtype=int)
                idxp[:k] = idx
                # conv1 weights gathered on OUTPUT channel; pad -> zero
                w1g = w1l[:, :, idxp].copy()  # [ci, s, np_s]
                w1g[:, :, k:] = 0
                m[f"w1g{s}"] = np.ascontiguousarray(
                    w1g.reshape(C // 128, 128, 9, np_s)
                ).astype(wdt)
                # conv2 weights gathered on INPUT channel; pad -> zero
                w2g = w2r[idxp].copy()  # [np_s, s, co]
                w2g[k:] = 0
                m[f"w2g{s}"] = np.ascontiguousarray(
                    w2g.reshape(slot_tiles[s], 128, 9, C)
                ).astype(wdt)
                sg = np.zeros(np_s, np.float32)
                tg = np.zeros(np_s, np.float32)
                sg[:k] = s1[idx]
                tg[:k] = t1[idx]
                m[f"s1vg{s}"] = sg.reshape(np_s, 1)
                m[f"t1vg{s}"] = tg.reshape(np_s, 1)
            in_maps.append(m)
    else:
        nc = _get_nc(("dense", mm_mode), mm_mode=mm_mode)
        s1v = (s1[None, :] * vector).astype(np.float32)
        t1v = (t1[None, :] * vector).astype(np.float32)
        w1l = _prep_weights(w1, mm_mode)
        w2l = _prep_weights(w2, mm_mode)
        xs = xp.reshape(NCORES, BPC, C, FLAT)
        in_maps = []
        for cid in range(NCORES):
            sl = slice(cid * BPC, (cid + 1) * BPC)
            in_maps.append(
                dict(
                    x=np.ascontiguousarray(xs[cid]),
                    mask=np.ascontiguousarray(mask_bf[sl]),
                    maskd=np.ascontiguousarray(maskd_bf[sl]),
                    w1=w1l,
                    w2=w2l,
                    s1v=np.ascontiguousarray(s1v[sl].reshape(BPC, C, 1)),
                    t1v=np.ascontiguousarray(t1v[sl].reshape(BPC, C, 1)),
                    s2=np.ascontiguousarray(s2.reshape(C, 1).astype(np.float32)),
                    t2=np.ascontiguousarray(t2.reshape(C, 1).astype(np.float32)),
                )
            )

    res = bass_utils.run_bass_kernel_spmd(
        nc, in_maps, core_ids=list(range(NCORES)), trace=TRACE
    )
    LAST_EXEC_NS = res.exec_time_ns
    LAST_TRACE = res.instructions_and_trace[1] if res.instructions_and_trace else None

    y = np.empty((B, C, FLAT), np.float32)
    if use_sparse:
        for cid in range(NCORES):
            for s in range(BPC):
                y[int(slots[s, cid])] = res.results[cid]["out"][s]
    else:
        for cid in range(NCORES):
            y[cid * BPC : (cid + 1) * BPC] = res.results[cid]["out"]
    return np.ascontiguousarray(
        y.reshape(B, C, PH, PW)[:, :, 1 : H + 1, 1 : W + 1]
    )


# revision 19
# speedup vs baseline: 1.3414x; 1.0269x over previous
"""Trainium2 Bass kernel for the sparse BasicBlock problem.

Math (masks and `vector` are binary in setup_inputs; verified at runtime):
    g   = x * mask_dilate
    c1  = conv3x3(g, w1)
    h   = relu(c1 * s1v + t1v) * mask      (BN1 affine folded with `vector`)
    c2  = conv3x3(h, w2)
    out = relu(x + (c2 * s2 + t2) * mask)

Layout: per image, channels on SBUF partitions, spatial flattened as a
zero-padded (H+2)x(W+2) row-major plane so a 3x3 conv is 9 shifted matmuls
accumulating in PSUM (shift = (dy-1)*(W+2) + (dx-1)). Matmuls run in fp32r
(full PE rate, ~1e-4 relative error).

Channel sparsity: `vector` zeroes ~half of conv1's output channels per image
(h == 0 there), so conv1 computes only the active channels (M-compaction) and
conv2 contracts only over them (K-compaction), via host-side gathered and
zero-padded per-image weights. One SPMD program is shared by all 8 cores, so
images are sorted by active-channel count and assigned so that each image
slot has a fixed channel-tile count across cores (max over cores).

Sharding: data-parallel over batch, 4 images per core on 8 cores.
"""

import sys
import types
from contextlib import ExitStack

sys.path.insert(0, "/opt/trn_rl_repo")

import ml_dtypes
import numpy as np

import concourse.bacc as bacc
import concourse.bass as bass
import concourse.mybir as mybir
import concourse.tile as tile
from concourse import bass_utils

# ----------------------------------------------------------------------------
# axon NTFF profiling hook shim (enables trace=True under axon)
# ----------------------------------------------------------------------------
_HOOK = {"hook": None}


def _install_axon_hooks():
    try:
        import antenv  # noqa: F401
    except ImportError:
        return
    if "antenv.axon_hooks" not in sys.modules:
        mod = types.ModuleType("antenv.axon_hooks")
        mod.set_axon_ntff_profile_hook = lambda h: _HOOK.__setitem__("hook", h)
        mod.get_axon_ntff_profile_hook = lambda: _HOOK["hook"]
        sys.modules["antenv.axon_hooks"] = mod
    if _HOOK["hook"] is None:
        try:
            from trn_agent_boot.trn_boot import _ntff_profile_via_ctypes

            sys.modules["antenv.axon_hooks"].set_axon_ntff_profile_hook(
                _ntff_profile_via_ctypes("/opt/axon/libaxon_pjrt.so")
            )
        except Exception:
            pass


_install_axon_hooks()
bass_utils.upload_artifacts = lambda tmpdir: tmpdir  # no S3 in this container

# ----------------------------------------------------------------------------
# problem constants (hardcoded per spec)
# ----------------------------------------------------------------------------
B, C, H, W = 32, 256, 56, 56
NCORES = 8
BPC = B // NCORES
EPS = 1e-5

TRACE = False
MM_MODE = "f32r"  # 'f32r' | 'bf16' | 'f32'
SPARSE = True
LAST_EXEC_NS = None
LAST_TRACE = None

F32 = mybir.dt.float32
BF16 = mybir.dt.bfloat16


def _chunks(total, maxw):
    """EVEN-width chunks <= maxw (fp32r needs an even moving dim; >=256 keeps
    full PE rate)."""
    assert total % 2 == 0, total
    n = -(-total // maxw)
    base = (total // n) & ~1
    rem = total - base * n
    out = []
    off = 0
    for i in range(n):
        w = base + (2 if i < rem // 2 else 0)
        out.append((off, w))
        off += w
    assert off == total
    return out


def _mdt(mm_mode):
    return {"f32r": mybir.dt.float32r, "bf16": BF16, "f32": F32}[mm_mode]


def build_nc(mm_mode=MM_MODE, bpc=BPC, c=C, h=H, w=W, slot_tiles=None):
    """Build the per-core SPMD Bass program.

    slot_tiles: None for the dense kernel, else per-image-slot channel-tile
    counts (e.g. (2, 2, 1, 1)) for the sparsity-specialized kernel.
    """
    PW, PH = w + 2, h + 2
    FLAT = PH * PW
    CT = c // 128
    NS = 9
    shifts = [(dy - 1) * PW + (dx - 1) for dy in range(3) for dx in range(3)]
    out_lo = PW + 1
    out_hi = h * PW + w
    span = out_hi - out_lo + 1
    chunks = [(out_lo + o, s) for (o, s) in _chunks(span, 464)]
    chunk_alloc = max(s for _, s in chunks)

    sparse = slot_tiles is not None
    if sparse:
        assert len(slot_tiles) == bpc
        max_nt = max(slot_tiles)
    mdt = _mdt(mm_mode)
    edt = F32 if mm_mode == "f32r" else mdt

    nc = bacc.Bacc("TRN2", debug=False, enable_asserts=False, num_devices=NCORES)

    # x / masks / out are passed HOST-PADDED to the (h+2)x(w+2) plane so every
    # large DMA is fully contiguous
    x_d = nc.dram_tensor("x", [bpc, c, FLAT], F32, kind="ExternalInput").ap()
    mask_d = nc.dram_tensor("mask", [bpc, FLAT], BF16, kind="ExternalInput").ap()
    maskd_d = nc.dram_tensor("maskd", [bpc, FLAT], BF16, kind="ExternalInput").ap()
    s2_d = nc.dram_tensor("s2", [c, 1], F32, kind="ExternalInput").ap()
    t2_d = nc.dram_tensor("t2", [c, 1], F32, kind="ExternalInput").ap()
    out_d = nc.dram_tensor("out", [bpc, c, FLAT], F32, kind="ExternalOutput").ap()
    if sparse:
        w1_d, w2_d, s1_d, t1_d = [], [], [], []
        for s, nt in enumerate(slot_tiles):
            np_s = 128 * nt
            w1_d.append(
                nc.dram_tensor(f"w1g{s}", [CT, 128, NS, np_s], mdt, kind="ExternalInput").ap()
            )
            w2_d.append(
                nc.dram_tensor(f"w2g{s}", [nt, 128, NS, c], mdt, kind="ExternalInput").ap()
            )
            s1_d.append(
                nc.dram_tensor(f"s1vg{s}", [np_s, 1], F32, kind="ExternalInput").ap()
            )
            t1_d.append(
                nc.dram_tensor(f"t1vg{s}", [np_s, 1], F32, kind="ExternalInput").ap()
            )
    else:
        w1s_d = nc.dram_tensor("w1", [CT, 128, NS, c], mdt, kind="ExternalInput").ap()
        w2s_d = nc.dram_tensor("w2", [CT, 128, NS, c], mdt, kind="ExternalInput").ap()
        s1v_d = nc.dram_tensor("s1v", [bpc, c, 1], F32, kind="ExternalInput").ap()
        t1v_d = nc.dram_tensor("t1v", [bpc, c, 1], F32, kind="ExternalInput").ap()

    Relu = mybir.ActivationFunctionType.Relu
    Ident = mybir.ActivationFunctionType.Identity

    with tile.TileContext(nc) as tc, ExitStack() as ctx:
        wpool = ctx.enter_context(tc.tile_pool(name="wpool", bufs=1))
        w1pool = ctx.enter_context(tc.tile_pool(name="w1pool", bufs=2))
        cpool = ctx.enter_context(tc.tile_pool(name="cpool", bufs=1))
        ppool = ctx.enter_context(tc.tile_pool(name="ppool", bufs=2))
        xpool = ctx.enter_context(tc.tile_pool(name="xpool", bufs=CT + 1))
        spool = ctx.enter_context(tc.tile_pool(name="spool", bufs=CT + 1))
        hpool = ctx.enter_context(
            tc.tile_pool(name="hpool", bufs=(max(2, max_nt) if sparse else CT))
        )
        mpool = ctx.enter_context(tc.tile_pool(name="mpool", bufs=2))
        mdpool = ctx.enter_context(tc.tile_pool(name="mdpool", bufs=2))
        epool = ctx.enter_context(tc.tile_pool(name="epool", bufs=8))
        pspool = ctx.enter_context(tc.tile_pool(name="psum", bufs=8, space="PSUM"))

        # bn2 params (shared)
        s2_sb = cpool.tile([128, CT, 1], F32)
        t2_sb = cpool.tile([128, CT, 1], F32)
        for co_t in range(CT):
            nc.scalar.dma_start(out=s2_sb[:, co_t], in_=s2_d[co_t * 128 : (co_t + 1) * 128])
            nc.scalar.dma_start(out=t2_sb[:, co_t], in_=t2_d[co_t * 128 : (co_t + 1) * 128])

        if not sparse:
            w1_sb = wpool.tile([128, CT, NS, c], mdt)
            w2_sb = wpool.tile([128, CT, NS, c], mdt)

        for i in range(bpc):
            nt = slot_tiles[i] if sparse else CT  # conv1 output tiles / conv2 K tiles
            np_i = 128 * nt

            # ---- masks: 1-row DMA into partition 0, then in-place broadcast ----
            maskd_pad = mdpool.tile([128, FLAT], BF16, tag="md", name=f"maskd{i}")
            nc.sync.dma_start(out=maskd_pad[0:1, :], in_=maskd_d[i : i + 1])
            nc.gpsimd.partition_broadcast(maskd_pad, maskd_pad[0:1, :])

            mask_pad = mpool.tile([128, FLAT], BF16, tag="m", name=f"mask{i}")
            nc.sync.dma_start(out=mask_pad[0:1, :], in_=mask_d[i : i + 1])

            # ---- x (padded, sync ring) and g = x * mask_dilate ----
            x_pad, g_pad = [], []
            for ci_t in range(CT):
                xt = xpool.tile([128, FLAT], F32, tag="x", name=f"x{i}_{ci_t}")
                nc.sync.dma_start(out=xt, in_=x_d[i, ci_t * 128 : (ci_t + 1) * 128])
                x_pad.append(xt)
                gt = spool.tile([128, FLAT], mdt, tag="scr", name=f"g{i}_{ci_t}")
                nc.vector.tensor_mul(gt, xt, maskd_pad)
                g_pad.append(gt)
            nc.gpsimd.partition_broadcast(mask_pad, mask_pad[0:1, :])

            # ---- weights for this image (scalar/HWDGE ring) ----
            if sparse:
                w1_sb = w1pool.tile([128, CT, NS, np_i], mdt, tag="w1g", name=f"w1g{i}")
                for ci_t in range(CT):
                    nc.scalar.dma_start(out=w1_sb[:, ci_t], in_=w1_d[i][ci_t])
                w2_sb = wpool.tile([128, nt, NS, c], mdt, tag="w2g", name=f"w2g{i}")
                for ci_t in range(nt):
                    nc.scalar.dma_start(out=w2_sb[:, ci_t], in_=w2_d[i][ci_t])
            elif i == 0:
                for ci_t in range(CT):
                    nc.scalar.dma_start(out=w1_sb[:, ci_t], in_=w1s_d[ci_t])
                    nc.scalar.dma_start(out=w2_sb[:, ci_t], in_=w2s_d[ci_t])

            # ---- folded bn1*vector params ----
            s1v_t = ppool.tile([128, nt, 1], F32, tag="s1v", name=f"s1v{i}")
            t1v_t = ppool.tile([128, nt, 1], F32, tag="t1v", name=f"t1v{i}")
            for co_t in range(nt):
                if sparse:
                    nc.scalar.dma_start(
                        out=s1v_t[:, co_t], in_=s1_d[i][co_t * 128 : (co_t + 1) * 128]
                    )
                    nc.scalar.dma_start(
                        out=t1v_t[:, co_t], in_=t1_d[i][co_t * 128 : (co_t + 1) * 128]
                    )
                else:
                    nc.scalar.dma_start(
                        out=s1v_t[:, co_t], in_=s1v_d[i, co_t * 128 : (co_t + 1) * 128]
                    )
                    nc.scalar.dma_start(
                        out=t1v_t[:, co_t], in_=t1v_d[i, co_t * 128 : (co_t + 1) * 128]
                    )

            # ---- conv1 -> h (active channels only in sparse mode) ----
            h_pad = []
            for co_t in range(nt):
                ht = hpool.tile([128, FLAT], mdt, tag="h", name=f"h{i}_{co_t}")
                nc.vector.tensor_scalar_mul(ht[:, 0:out_lo], x_pad[0][:, 0:out_lo], 0.0)
                nc.vector.tensor_scalar_mul(
                    ht[:, out_hi + 1 : FLAT], x_pad[0][:, out_hi + 1 : FLAT], 0.0
                )
                h_pad.append(ht)

            def epi1(co_t, off, wd, ps):
                r = epool.tile([128, chunk_alloc], edt, tag="e", name=f"r{i}_{co_t}_{off}")
                nc.scalar.activation(
                    r[:, :wd], ps[:, :wd], Relu, bias=t1v_t[:, co_t], scale=s1v_t[:, co_t]
                )
                nc.vector.tensor_mul(
                    h_pad[co_t][:, off : off + wd], r[:, :wd], mask_pad[:, off : off + wd]
                )

            if i == 0:
                # first image: ci-outer / chunk-inner over all 7 PSUM banks, so
                # the first 9*7 matmuls depend only on g_pad[0] (hides the
                # second x-tile DMA + g-multiply latency at kernel start)
                for co_t in range(nt):
                    pss = [
                        pspool.tile([128, chunk_alloc], F32, tag="ps", name=f"ps1h_{co_t}_{ck}")
                        for ck in range(len(chunks))
                    ]
                    k, nk = 0, CT * NS
                    for ci_t in range(CT):
                        for s in range(NS):
                            for ck, (off, wd) in enumerate(chunks):
                                nc.tensor.matmul(
                                    pss[ck][:, :wd],
                                    w1_sb[:, ci_t, s, co_t * 128 : co_t * 128 + 128],
                                    g_pad[ci_t][:, off + shifts[s] : off + shifts[s] + wd],
                                    start=(k == 0),
                                    stop=(k == nk - 1),
                                )
                            k += 1
                    for ck, (off, wd) in enumerate(chunks):
                        epi1(co_t, off, wd, pss[ck])
            else:
                for co_t in range(nt):
                    for off, wd in chunks:
                        ps = pspool.tile([128, chunk_alloc], F32, tag="ps", name=f"ps1_{i}_{co_t}_{off}")
                        k, nk = 0, CT * NS
                        for ci_t in range(CT):
                            for s in range(NS):
                                nc.tensor.matmul(
                                    ps[:, :wd],
                                    w1_sb[:, ci_t, s, co_t * 128 : co_t * 128 + 128],
                                    g_pad[ci_t][:, off + shifts[s] : off + shifts[s] + wd],
                                    start=(k == 0),
                                    stop=(k == nk - 1),
                                )
                                k += 1
                        epi1(co_t, off, wd, ps)

            # ---- conv2 -> out ----
            out_t = []
            for ct in range(CT):
                ot = spool.tile([128, FLAT], F32, tag="scr", name=f"o{i}_{ct}")
                nc.vector.memset(ot[:, 0:out_lo], 0.0)
                nc.vector.memset(ot[:, out_hi + 1 : FLAT], 0.0)
                out_t.append(ot)
            for co_t in range(CT):
                for off, wd in chunks:
                    ps = pspool.tile([128, chunk_alloc], F32, tag="ps", name=f"ps2_{i}_{co_t}_{off}")
                    k, nk = 0, nt * NS
                    for ci_t in range(nt):
                        for s in range(NS):
                            nc.tensor.matmul(
                                ps[:, :wd],
                                w2_sb[:, ci_t, s, co_t * 128 : co_t * 128 + 128],
                                h_pad[ci_t][:, off + shifts[s] : off + shifts[s] + wd],
                                start=(k == 0),
                                stop=(k == nk - 1),
                            )
                            k += 1
                    e = epool.tile([128, chunk_alloc], F32, tag="e", name=f"e{i}_{co_t}_{off}")
                    nc.scalar.activation(
                        e[:, :wd], ps[:, :wd], Ident, bias=t2_sb[:, co_t], scale=s2_sb[:, co_t]
                    )
                    nc.vector.tensor_mul(e[:, :wd], e[:, :wd], mask_pad[:, off : off + wd])
                    dst = out_t[co_t][:, off : off + wd]
                    nc.vector.tensor_add(dst, e[:, :wd], x_pad[co_t][:, off : off + wd])
                    nc.scalar.activation(dst, dst, Relu)

            for co_t in range(CT):
                eng = nc.sync if co_t == 0 else nc.scalar
                eng.dma_start(
                    out=out_d[i, co_t * 128 : (co_t + 1) * 128], in_=out_t[co_t]
                )

    nc.compile()
    return nc


# ----------------------------------------------------------------------------
# host-side prep + execution
# ----------------------------------------------------------------------------
_NC_CACHE = {}


def _get_nc(key, **kw):
    if key not in _NC_CACHE:
        _NC_CACHE[key] = build_nc(**kw)
    return _NC_CACHE[key]


def _wt_np(mm_mode):
    return ml_dtypes.bfloat16 if mm_mode == "bf16" else np.float32


def _prep_weights(wt, mm_mode, c=C):
    # [co, ci, 3, 3] -> [ci_t, ci, s, co] with s = dy*3+dx
    t = np.ascontiguousarray(wt.transpose(1, 2, 3, 0).reshape(c // 128, 128, 9, c))
    return t.astype(_wt_np(mm_mode))


def kernel(**inputs):
    global LAST_EXEC_NS, LAST_TRACE
    x = np.asarray(inputs["x"], dtype=np.float32)
    mask = np.asarray(inputs["mask"], dtype=np.float32).reshape(B, H * W)
    maskd = np.asarray(inputs["mask_dilate"], dtype=np.float32).reshape(B, H * W)
    vector = np.asarray(inputs["vector"], dtype=np.float32)
    w1 = np.asarray(inputs["conv1_w"], dtype=np.float32)
    w2 = np.asarray(inputs["conv2_w"], dtype=np.float32)

    s1 = np.asarray(inputs["bn1_g"]) / np.sqrt(np.asarray(inputs["bn1_v"]) + EPS)
    t1 = np.asarray(inputs["bn1_b"]) - np.asarray(inputs["bn1_m"]) * s1
    s2 = np.asarray(inputs["bn2_g"]) / np.sqrt(np.asarray(inputs["bn2_v"]) + EPS)
    t2 = np.asarray(inputs["bn2_b"]) - np.asarray(inputs["bn2_m"]) * s2
    s1, t1 = s1.astype(np.float32), t1.astype(np.float32)

    binary = lambda a: bool(np.isin(a, (0.0, 1.0)).all())  # noqa: E731
    masks_binary = binary(mask) and binary(maskd)
    assert (vector >= 0).all() and masks_binary, (
        "kernel specialized for setup_inputs-style binary masks / nonneg vector"
    )
    use_sparse = SPARSE and binary(vector)

    # host-pad x and masks to the (H+2)x(W+2) plane => contiguous device DMAs
    PW, PH = W + 2, H + 2
    FLAT = PH * PW
    xp = np.zeros((B, C, PH, PW), np.float32)
    xp[:, :, 1 : H + 1, 1 : W + 1] = x
    xp = xp.reshape(B, C, FLAT)
    mask_bf = np.zeros((B, PH, PW), ml_dtypes.bfloat16)
    mask_bf[:, 1 : H + 1, 1 : W + 1] = mask.reshape(B, H, W)
    mask_bf = mask_bf.reshape(B, FLAT)
    maskd_bf = np.zeros((B, PH, PW), ml_dtypes.bfloat16)
    maskd_bf[:, 1 : H + 1, 1 : W + 1] = maskd.reshape(B, H, W)
    maskd_bf = maskd_bf.reshape(B, FLAT)
    mm_mode = MM_MODE
    wdt = _wt_np(mm_mode)

    if use_sparse:
        nact = vector.sum(1).astype(int)
        order = np.argsort(-nact, kind="stable")
        slots = order.reshape(BPC, NCORES)  # [slot, core] -> original image idx
        # put a cheap (low tile-count) slot first so image 0's setup is light,
        # then the heavy slots
        rank = np.argsort([nact[slots[s]].max() for s in range(BPC)])
        perm = []
        light, heavy = list(rank), []
        if BPC >= 2:
            light, heavy = [rank[0]], list(rank[1:][::-1])
        perm = light + heavy
        slots = slots[perm]
        slot_tiles = tuple(
            max(1, int(np.ceil(nact[slots[s]].max() / 128))) for s in range(BPC)
        )
        if sum(slot_tiles) >= BPC * (C // 128):
            use_sparse = False  # no win; fall back to shared-weight dense kernel

    if use_sparse:
        nc = _get_nc(("sparse", mm_mode, slot_tiles), mm_mode=mm_mode, slot_tiles=slot_tiles)
        # full lhsT layouts to gather from
        w1l = w1.transpose(1, 2, 3, 0).reshape(C, 9, C)  # [ci, s, co]
        w2r = w2.transpose(1, 2, 3, 0).reshape(C, 9, C)  # [ci, s, co] rows = conv2 input ch
        in_maps = []
        for cid in range(NCORES):
            imgs = [int(slots[s, cid]) for s in range(BPC)]
            m = dict(
                x=np.ascontiguousarray(xp[imgs]),
                mask=np.ascontiguousarray(mask_bf[imgs]),
                maskd=np.ascontiguousarray(maskd_bf[imgs]),
                s2=np.ascontiguousarray(s2.reshape(C, 1).astype(np.float32)),
                t2=np.ascontiguousarray(t2.reshape(C, 1).astype(np.float32)),
            )
            for s, b in enumerate(imgs):
                np_s = 128 * slot_tiles[s]
                idx = np.where(vector[b] > 0)[0]
                k = len(idx)
                idxp = np.zeros(np_s, dtype=int)
                idxp[:k] = idx
                # conv1 weights gathered on OUTPUT channel; pad -> zero
                w1g = w1l[:, :, idxp].copy()  # [ci, s, np_s]
                w1g[:, :, k:] = 0
                m[f"w1g{s}"] = np.ascontiguousarray(
                    w1g.reshape(C // 128, 128, 9, np_s)
                ).astype(wdt)
                # conv2 weights gathered on INPUT channel; pad -> zero
                w2g = w2r[idxp].copy()  # [np_s, s, co]
                w2g[k:] = 0
                m[f"w2g{s}"] = np.ascontiguousarray(
                    w2g.reshape(slot_tiles[s], 128, 9, C)
                ).astype(wdt)
                sg = np.zeros(np_s, np.float32)
                tg = np.zeros(np_s, np.float32)
                sg[:k] = s1[idx]
                tg[:k] = t1[idx]
                m[f"s1vg{s}"] = sg.reshape(np_s, 1)
                m[f"t1vg{s}"] = tg.reshape(np_s, 1)
            in_maps.append(m)
    else:
        nc = _get_nc(("dense", mm_mode), mm_mode=mm_mode)
        s1v = (s1[None, :] * vector).astype(np.float32)
        t1v = (t1[None, :] * vector).astype(np.float32)
        w1l = _prep_weights(w1, mm_mode)
        w2l = _prep_weights(w2, mm_mode)
        xs = xp.reshape(NCORES, BPC, C, FLAT)
        in_maps = []
        for cid in range(NCORES):
            sl = slice(cid * BPC, (cid + 1) * BPC)
            in_maps.append(
                dict(
                    x=np.ascontiguousarray(xs[cid]),
                    mask=np.ascontiguousarray(mask_bf[sl]),
                    maskd=np.ascontiguousarray(maskd_bf[sl]),
                    w1=w1l,
                    w2=w2l,
                    s1v=np.ascontiguousarray(s1v[sl].reshape(BPC, C, 1)),
                    t1v=np.ascontiguousarray(t1v[sl].reshape(BPC, C, 1)),
                    s2=np.ascontiguousarray(s2.reshape(C, 1).astype(np.float32)),
                    t2=np.ascontiguousarray(t2.reshape(C, 1).astype(np.float32)),
                )
            )

    res = bass_utils.run_bass_kernel_spmd(
        nc, in_maps, core_ids=list(range(NCORES)), trace=TRACE
    )
    LAST_EXEC_NS = res.exec_time_ns
    LAST_TRACE = res.instructions_and_trace[1] if res.instructions_and_trace else None

    y = np.empty((B, C, FLAT), np.float32)
    if use_sparse:
        for cid in range(NCORES):
            for s in range(BPC):
                y[int(slots[s, cid])] = res.results[cid]["out"][s]
    else:
        for cid in range(NCORES):
            y[cid * BPC : (cid + 1) * BPC] = res.results[cid]["out"]
    return np.ascontiguousarray(
        y.reshape(B, C, PH, PW)[:, :, 1 : H + 1, 1 : W + 1]
    )


# revision 20
# speedup vs baseline: 1.3687x; 1.0203x over previous
"""Trainium2 Bass kernel for the sparse BasicBlock problem.

Math (masks and `vector` are binary in setup_inputs; verified at runtime):
    g   = x * mask_dilate
    c1  = conv3x3(g, w1)
    h   = relu(c1 * s1v + t1v) * mask      (BN1 affine folded with `vector`)
    c2  = conv3x3(h, w2)
    out = relu(x + (c2 * s2 + t2) * mask)

Layout: per image, channels on SBUF partitions, spatial flattened as a
zero-padded (H+2)x(W+2) row-major plane so a 3x3 conv is 9 shifted matmuls
accumulating in PSUM (shift = (dy-1)*(W+2) + (dx-1)). Matmuls run in fp32r
(full PE rate, ~1e-4 relative error).

Channel sparsity: `vector` zeroes ~half of conv1's output channels per image
(h == 0 there), so conv1 computes only the active channels (M-compaction) and
conv2 contracts only over them (K-compaction), via host-side gathered and
zero-padded per-image weights. One SPMD program is shared by all 8 cores, so
images are sorted by active-channel count and assigned so that each image
slot has a fixed channel-tile count across cores (max over cores).

Sharding: data-parallel over batch, 4 images per core on 8 cores.
"""

import sys
import types
from contextlib import ExitStack

sys.path.insert(0, "/opt/trn_rl_repo")

import ml_dtypes
import numpy as np

import concourse.bacc as bacc
import concourse.bass as bass
import concourse.mybir as mybir
import concourse.tile as tile
from concourse import bass_utils

# ----------------------------------------------------------------------------
# axon NTFF profiling hook shim (enables trace=True under axon)
# ----------------------------------------------------------------------------
_HOOK = {"hook": None}


def _install_axon_hooks():
    try:
        import antenv  # noqa: F401
    except ImportError:
        return
    if "antenv.axon_hooks" not in sys.modules:
        mod = types.ModuleType("antenv.axon_hooks")
        mod.set_axon_ntff_profile_hook = lambda h: _HOOK.__setitem__("hook", h)
        mod.get_axon_ntff_profile_hook = lambda: _HOOK["hook"]
        sys.modules["antenv.axon_hooks"] = mod
    if _HOOK["hook"] is None:
        try:
            from trn_agent_boot.trn_boot import _ntff_profile_via_ctypes

            sys.modules["antenv.axon_hooks"].set_axon_ntff_profile_hook(
                _ntff_profile_via_ctypes("/opt/axon/libaxon_pjrt.so")
            )
        except Exception:
            pass


_install_axon_hooks()
bass_utils.upload_artifacts = lambda tmpdir: tmpdir  # no S3 in this container

# ----------------------------------------------------------------------------
# problem constants (hardcoded per spec)
# ----------------------------------------------------------------------------
B, C, H, W = 32, 256, 56, 56
NCORES = 8
BPC = B // NCORES
EPS = 1e-5

TRACE = False
MM_MODE = "f32r"  # 'f32r' | 'bf16' | 'f32'
SPARSE = True
LAST_EXEC_NS = None
LAST_TRACE = None

F32 = mybir.dt.float32
BF16 = mybir.dt.bfloat16


def _chunks(total, maxw):
    """EVEN-width chunks <= maxw (fp32r needs an even moving dim; >=256 keeps
    full PE rate)."""
    assert total % 2 == 0, total
    n = -(-total // maxw)
    base = (total // n) & ~1
    rem = total - base * n
    out = []
    off = 0
    for i in range(n):
        w = base + (2 if i < rem // 2 else 0)
        out.append((off, w))
        off += w
    assert off == total
    return out


def _mdt(mm_mode):
    return {"f32r": mybir.dt.float32r, "bf16": BF16, "f32": F32}[mm_mode]


def build_nc(mm_mode=MM_MODE, bpc=BPC, c=C, h=H, w=W, slot_tiles=None):
    """Build the per-core SPMD Bass program.

    slot_tiles: None for the dense kernel, else per-image-slot channel-tile
    counts (e.g. (2, 2, 1, 1)) for the sparsity-specialized kernel.
    """
    PW, PH = w + 2, h + 2
    FLAT = PH * PW
    CT = c // 128
    NS = 9
    shifts = [(dy - 1) * PW + (dx - 1) for dy in range(3) for dx in range(3)]
    out_lo = PW + 1
    out_hi = h * PW + w
    span = out_hi - out_lo + 1
    chunks = [(out_lo + o, s) for (o, s) in _chunks(span, 464)]
    chunk_alloc = max(s for _, s in chunks)

    sparse = slot_tiles is not None
    if sparse:
        assert len(slot_tiles) == bpc
        max_nt = max(slot_tiles)
    mdt = _mdt(mm_mode)
    edt = F32 if mm_mode == "f32r" else mdt

    nc = bacc.Bacc("TRN2", debug=False, enable_asserts=False, num_devices=NCORES)

    # x / masks / out are passed HOST-PADDED to the (h+2)x(w+2) plane so every
    # large DMA is fully contiguous
    x_d = nc.dram_tensor("x", [bpc, c, FLAT], F32, kind="ExternalInput").ap()
    mask_d = nc.dram_tensor("mask", [bpc, FLAT], BF16, kind="ExternalInput").ap()
    maskd_d = nc.dram_tensor("maskd", [bpc, FLAT], BF16, kind="ExternalInput").ap()
    s2_d = nc.dram_tensor("s2", [c, 1], F32, kind="ExternalInput").ap()
    t2_d = nc.dram_tensor("t2", [c, 1], F32, kind="ExternalInput").ap()
    out_d = nc.dram_tensor("out", [bpc, c, FLAT], F32, kind="ExternalOutput").ap()
    if sparse:
        w1_d, w2_d, s1_d, t1_d = [], [], [], []
        for s, nt in enumerate(slot_tiles):
            np_s = 128 * nt
            w1_d.append(
                nc.dram_tensor(f"w1g{s}", [CT, 128, NS, np_s], mdt, kind="ExternalInput").ap()
            )
            w2_d.append(
                nc.dram_tensor(f"w2g{s}", [nt, 128, NS, c], mdt, kind="ExternalInput").ap()
            )
            s1_d.append(
                nc.dram_tensor(f"s1vg{s}", [np_s, 1], F32, kind="ExternalInput").ap()
            )
            t1_d.append(
                nc.dram_tensor(f"t1vg{s}", [np_s, 1], F32, kind="ExternalInput").ap()
            )
    else:
        w1s_d = nc.dram_tensor("w1", [CT, 128, NS, c], mdt, kind="ExternalInput").ap()
        w2s_d = nc.dram_tensor("w2", [CT, 128, NS, c], mdt, kind="ExternalInput").ap()
        s1v_d = nc.dram_tensor("s1v", [bpc, c, 1], F32, kind="ExternalInput").ap()
        t1v_d = nc.dram_tensor("t1v", [bpc, c, 1], F32, kind="ExternalInput").ap()

    Relu = mybir.ActivationFunctionType.Relu
    Ident = mybir.ActivationFunctionType.Identity

    with tile.TileContext(nc) as tc, ExitStack() as ctx:
        wpool = ctx.enter_context(tc.tile_pool(name="wpool", bufs=1))
        w1pool = ctx.enter_context(tc.tile_pool(name="w1pool", bufs=2))
        cpool = ctx.enter_context(tc.tile_pool(name="cpool", bufs=1))
        ppool = ctx.enter_context(tc.tile_pool(name="ppool", bufs=2))
        xpool = ctx.enter_context(tc.tile_pool(name="xpool", bufs=CT + 1))
        spool = ctx.enter_context(tc.tile_pool(name="spool", bufs=CT + 1))
        hpool = ctx.enter_context(
            tc.tile_pool(name="hpool", bufs=(max(2, max_nt) if sparse else CT))
        )
        mpool = ctx.enter_context(tc.tile_pool(name="mpool", bufs=2))
        mdpool = ctx.enter_context(tc.tile_pool(name="mdpool", bufs=2))
        epool = ctx.enter_context(tc.tile_pool(name="epool", bufs=8))
        pspool = ctx.enter_context(tc.tile_pool(name="psum", bufs=8, space="PSUM"))

        # bn2 params (shared)
        s2_sb = cpool.tile([128, CT, 1], F32)
        t2_sb = cpool.tile([128, CT, 1], F32)
        for co_t in range(CT):
            nc.scalar.dma_start(out=s2_sb[:, co_t], in_=s2_d[co_t * 128 : (co_t + 1) * 128])
            nc.scalar.dma_start(out=t2_sb[:, co_t], in_=t2_d[co_t * 128 : (co_t + 1) * 128])

        if not sparse:
            w1_sb = wpool.tile([128, CT, NS, c], mdt)
            w2_sb = wpool.tile([128, CT, NS, c], mdt)

        for i in range(bpc):
            nt = slot_tiles[i] if sparse else CT  # conv1 output tiles / conv2 K tiles
            np_i = 128 * nt

            # ---- masks: 1-row DMA into partition 0, then in-place broadcast ----
            maskd_pad = mdpool.tile([128, FLAT], BF16, tag="md", name=f"maskd{i}")
            nc.sync.dma_start(out=maskd_pad[0:1, :], in_=maskd_d[i : i + 1])
            nc.gpsimd.partition_broadcast(maskd_pad, maskd_pad[0:1, :])

            mask_pad = mpool.tile([128, FLAT], BF16, tag="m", name=f"mask{i}")
            nc.sync.dma_start(out=mask_pad[0:1, :], in_=mask_d[i : i + 1])

            # ---- x (padded, sync ring) and g = x * mask_dilate ----
            x_pad, g_pad = [], []
            for ci_t in range(CT):
                xt = xpool.tile([128, FLAT], F32, tag="x", name=f"x{i}_{ci_t}")
                nc.sync.dma_start(out=xt, in_=x_d[i, ci_t * 128 : (ci_t + 1) * 128])
                x_pad.append(xt)
                gt = spool.tile([128, FLAT], mdt, tag="scr", name=f"g{i}_{ci_t}")
                nc.vector.tensor_mul(gt, xt, maskd_pad)
                g_pad.append(gt)
            nc.gpsimd.partition_broadcast(mask_pad, mask_pad[0:1, :])

            # ---- weights for this image (scalar/HWDGE ring) ----
            if sparse:
                w1_sb = w1pool.tile([128, CT, NS, np_i], mdt, tag="w1g", name=f"w1g{i}")
                for ci_t in range(CT):
                    nc.scalar.dma_start(out=w1_sb[:, ci_t], in_=w1_d[i][ci_t])
                w2_sb = wpool.tile([128, nt, NS, c], mdt, tag="w2g", name=f"w2g{i}")
                for ci_t in range(nt):
                    nc.scalar.dma_start(out=w2_sb[:, ci_t], in_=w2_d[i][ci_t])
            elif i == 0:
                for ci_t in range(CT):
                    nc.scalar.dma_start(out=w1_sb[:, ci_t], in_=w1s_d[ci_t])
                    nc.scalar.dma_start(out=w2_sb[:, ci_t], in_=w2s_d[ci_t])

            # ---- folded bn1*vector params ----
            s1v_t = ppool.tile([128, nt, 1], F32, tag="s1v", name=f"s1v{i}")
            t1v_t = ppool.tile([128, nt, 1], F32, tag="t1v", name=f"t1v{i}")
            for co_t in range(nt):
                if sparse:
                    nc.scalar.dma_start(
                        out=s1v_t[:, co_t], in_=s1_d[i][co_t * 128 : (co_t + 1) * 128]
                    )
                    nc.scalar.dma_start(
                        out=t1v_t[:, co_t], in_=t1_d[i][co_t * 128 : (co_t + 1) * 128]
                    )
                else:
                    nc.scalar.dma_start(
                        out=s1v_t[:, co_t], in_=s1v_d[i, co_t * 128 : (co_t + 1) * 128]
                    )
                    nc.scalar.dma_start(
                        out=t1v_t[:, co_t], in_=t1v_d[i, co_t * 128 : (co_t + 1) * 128]
                    )

            # ---- conv1 -> h (active channels only in sparse mode) ----
            h_pad = []
            for co_t in range(nt):
                ht = hpool.tile([128, FLAT], mdt, tag="h", name=f"h{i}_{co_t}")
                nc.vector.tensor_scalar_mul(ht[:, 0:out_lo], x_pad[0][:, 0:out_lo], 0.0)
                nc.vector.tensor_scalar_mul(
                    ht[:, out_hi + 1 : FLAT], x_pad[0][:, out_hi + 1 : FLAT], 0.0
                )
                h_pad.append(ht)

            def epi1(co_t, off, wd, ps):
                r = epool.tile([128, chunk_alloc], edt, tag="e", name=f"r{i}_{co_t}_{off}")
                nc.scalar.activation(
                    r[:, :wd], ps[:, :wd], Relu, bias=t1v_t[:, co_t], scale=s1v_t[:, co_t]
                )
                nc.vector.tensor_mul(
                    h_pad[co_t][:, off : off + wd], r[:, :wd], mask_pad[:, off : off + wd]
                )

            # weight-stationary grouped accumulation: per co-tile, chunks are
            # processed in 2 groups; within a group the (ci,shift) loop is
            # outer so each weight tile is loaded once per group, and group A's
            # epilogues overlap group B's matmuls
            def grouped_conv(w_sb, n_k, rhs, n_out, epi, pfx):
                groups = [
                    list(enumerate(chunks))[0 : len(chunks) // 2],
                    list(enumerate(chunks))[len(chunks) // 2 :],
                ]
                for co_t in range(n_out):
                    for gi, grp in enumerate(groups):
                        pss = {
                            ck: pspool.tile(
                                [128, chunk_alloc], F32, tag="ps", name=f"{pfx}_{co_t}_{ck}"
                            )
                            for ck, _ in grp
                        }
                        k, nk = 0, n_k * NS
                        for ci_t in range(n_k):
                            for s in range(NS):
                                for ck, (off, wd) in grp:
                                    nc.tensor.matmul(
                                        pss[ck][:, :wd],
                                        w_sb[:, ci_t, s, co_t * 128 : co_t * 128 + 128],
                                        rhs[ci_t][:, off + shifts[s] : off + shifts[s] + wd],
                                        start=(k == 0),
                                        stop=(k == nk - 1),
                                    )
                                k += 1
                        for ck, (off, wd) in grp:
                            epi(co_t, off, wd, pss[ck])

            def epi1(co_t, off, wd, ps):
                r = epool.tile([128, chunk_alloc], edt, tag="e", name=f"r{i}_{co_t}_{off}")
                nc.scalar.activation(
                    r[:, :wd], ps[:, :wd], Relu, bias=t1v_t[:, co_t], scale=s1v_t[:, co_t]
                )
                nc.vector.tensor_mul(
                    h_pad[co_t][:, off : off + wd], r[:, :wd], mask_pad[:, off : off + wd]
                )

            grouped_conv(w1_sb, CT, g_pad, nt, epi1, f"ps1_{i}")

            # ---- conv2 -> out ----
            out_t = []
            for ct in range(CT):
                ot = spool.tile([128, FLAT], F32, tag="scr", name=f"o{i}_{ct}")
                nc.vector.memset(ot[:, 0:out_lo], 0.0)
                nc.vector.memset(ot[:, out_hi + 1 : FLAT], 0.0)
                out_t.append(ot)
            def epi2(co_t, off, wd, ps):
                e = epool.tile([128, chunk_alloc], F32, tag="e", name=f"e{i}_{co_t}_{off}")
                nc.scalar.activation(
                    e[:, :wd], ps[:, :wd], Ident, bias=t2_sb[:, co_t], scale=s2_sb[:, co_t]
                )
                nc.vector.tensor_mul(e[:, :wd], e[:, :wd], mask_pad[:, off : off + wd])
                dst = out_t[co_t][:, off : off + wd]
                nc.vector.tensor_add(dst, e[:, :wd], x_pad[co_t][:, off : off + wd])
                nc.scalar.activation(dst, dst, Relu)

            grouped_conv(w2_sb, nt, h_pad, CT, epi2, f"ps2_{i}")

            for co_t in range(CT):
                eng = nc.sync if co_t == 0 else nc.scalar
                eng.dma_start(
                    out=out_d[i, co_t * 128 : (co_t + 1) * 128], in_=out_t[co_t]
                )

    nc.compile()
    return nc


# ----------------------------------------------------------------------------
# host-side prep + execution
# ----------------------------------------------------------------------------
_NC_CACHE = {}


def _get_nc(key, **kw):
    if key not in _NC_CACHE:
        _NC_CACHE[key] = build_nc(**kw)
    return _NC_CACHE[key]


def _wt_np(mm_mode):
    return ml_dtypes.bfloat16 if mm_mode == "bf16" else np.float32


def _prep_weights(wt, mm_mode, c=C):
    # [co, ci, 3, 3] -> [ci_t, ci, s, co] with s = dy*3+dx
    t = np.ascontiguousarray(wt.transpose(1, 2, 3, 0).reshape(c // 128, 128, 9, c))
    return t.astype(_wt_np(mm_mode))


def kernel(**inputs):
    global LAST_EXEC_NS, LAST_TRACE
    x = np.asarray(inputs["x"], dtype=np.float32)
    mask = np.asarray(inputs["mask"], dtype=np.float32).reshape(B, H * W)
    maskd = np.asarray(inputs["mask_dilate"], dtype=np.float32).reshape(B, H * W)
    vector = np.asarray(inputs["vector"], dtype=np.float32)
    w1 = np.asarray(inputs["conv1_w"], dtype=np.float32)
    w2 = np.asarray(inputs["conv2_w"], dtype=np.float32)

    s1 = np.asarray(inputs["bn1_g"]) / np.sqrt(np.asarray(inputs["bn1_v"]) + EPS)
    t1 = np.asarray(inputs["bn1_b"]) - np.asarray(inputs["bn1_m"]) * s1
    s2 = np.asarray(inputs["bn2_g"]) / np.sqrt(np.asarray(inputs["bn2_v"]) + EPS)
    t2 = np.asarray(inputs["bn2_b"]) - np.asarray(inputs["bn2_m"]) * s2
    s1, t1 = s1.astype(np.float32), t1.astype(np.float32)

    binary = lambda a: bool(np.isin(a, (0.0, 1.0)).all())  # noqa: E731
    masks_binary = binary(mask) and binary(maskd)
    assert (vector >= 0).all() and masks_binary, (
        "kernel specialized for setup_inputs-style binary masks / nonneg vector"
    )
    use_sparse = SPARSE and binary(vector)

    # host-pad x and masks to the (H+2)x(W+2) plane => contiguous device DMAs
    PW, PH = W + 2, H + 2
    FLAT = PH * PW
    xp = np.zeros((B, C, PH, PW), np.float32)
    xp[:, :, 1 : H + 1, 1 : W + 1] = x
    xp = xp.reshape(B, C, FLAT)
    mask_bf = np.zeros((B, PH, PW), ml_dtypes.bfloat16)
    mask_bf[:, 1 : H + 1, 1 : W + 1] = mask.reshape(B, H, W)
    mask_bf = mask_bf.reshape(B, FLAT)
    maskd_bf = np.zeros((B, PH, PW), ml_dtypes.bfloat16)
    maskd_bf[:, 1 : H + 1, 1 : W + 1] = maskd.reshape(B, H, W)
    maskd_bf = maskd_bf.reshape(B, FLAT)
    mm_mode = MM_MODE
    wdt = _wt_np(mm_mode)

    if use_sparse:
        nact = vector.sum(1).astype(int)
        order = np.argsort(-nact, kind="stable")
        slots = order.reshape(BPC, NCORES)  # [slot, core] -> original image idx
        # put a cheap (low tile-count) slot first so image 0's setup is light,
        # then the heavy slots
        rank = np.argsort([nact[slots[s]].max() for s in range(BPC)])
        perm = []
        light, heavy = list(rank), []
        if BPC >= 2:
            light, heavy = [rank[0]], list(rank[1:][::-1])
        perm = light + heavy
        slots = slots[perm]
        slot_tiles = tuple(
            max(1, int(np.ceil(nact[slots[s]].max() / 128))) for s in range(BPC)
        )
        if sum(slot_tiles) >= BPC * (C // 128):
            use_sparse = False  # no win; fall back to shared-weight dense kernel

    if use_sparse:
        nc = _get_nc(("sparse", mm_mode, slot_tiles), mm_mode=mm_mode, slot_tiles=slot_tiles)
        # full lhsT layouts to gather from
        w1l = w1.transpose(1, 2, 3, 0).reshape(C, 9, C)  # [ci, s, co]
        w2r = w2.transpose(1, 2, 3, 0).reshape(C, 9, C)  # [ci, s, co] rows = conv2 input ch
        in_maps = []
        for cid in range(NCORES):
            imgs = [int(slots[s, cid]) for s in range(BPC)]
            m = dict(
                x=np.ascontiguousarray(xp[imgs]),
                mask=np.ascontiguousarray(mask_bf[imgs]),
                maskd=np.ascontiguousarray(maskd_bf[imgs]),
                s2=np.ascontiguousarray(s2.reshape(C, 1).astype(np.float32)),
                t2=np.ascontiguousarray(t2.reshape(C, 1).astype(np.float32)),
            )
            for s, b in enumerate(imgs):
                np_s = 128 * slot_tiles[s]
                idx = np.where(vector[b] > 0)[0]
                k = len(idx)
                idxp = np.zeros(np_s, dtype=int)
                idxp[:k] = idx
                # conv1 weights gathered on OUTPUT channel; pad -> zero
                w1g = w1l[:, :, idxp].copy()  # [ci, s, np_s]
                w1g[:, :, k:] = 0
                m[f"w1g{s}"] = np.ascontiguousarray(
                    w1g.reshape(C // 128, 128, 9, np_s)
                ).astype(wdt)
                # conv2 weights gathered on INPUT channel; pad -> zero
                w2g = w2r[idxp].copy()  # [np_s, s, co]
                w2g[k:] = 0
                m[f"w2g{s}"] = np.ascontiguousarray(
                    w2g.reshape(slot_tiles[s], 128, 9, C)
                ).astype(wdt)
                sg = np.zeros(np_s, np.float32)
                tg = np.zeros(np_s, np.float32)
                sg[:k] = s1[idx]
                tg[:k] = t1[idx]
                m[f"s1vg{s}"] = sg.reshape(np_s, 1)
                m[f"t1vg{s}"] = tg.reshape(np_s, 1)
            in_maps.append(m)
    else:
        nc = _get_nc(("dense", mm_mode), mm_mode=mm_mode)
        s1v = (s1[None, :] * vector).astype(np.float32)
        t1v = (t1[None, :] * vector).astype(np.float32)
        w1l = _prep_weights(w1, mm_mode)
        w2l = _prep_weights(w2, mm_mode)
        xs = xp.reshape(NCORES, BPC, C, FLAT)
        in_maps = []
        for cid in range(NCORES):
            sl = slice(cid * BPC, (cid + 1) * BPC)
            in_maps.append(
                dict(
                    x=np.ascontiguousarray(xs[cid]),
                    mask=np.ascontiguousarray(mask_bf[sl]),
                    maskd=np.ascontiguousarray(maskd_bf[sl]),
                    w1=w1l,
                    w2=w2l,
                    s1v=np.ascontiguousarray(s1v[sl].reshape(BPC, C, 1)),
                    t1v=np.ascontiguousarray(t1v[sl].reshape(BPC, C, 1)),
                    s2=np.ascontiguousarray(s2.reshape(C, 1).astype(np.float32)),
                    t2=np.ascontiguousarray(t2.reshape(C, 1).astype(np.float32)),
                )
            )

    res = bass_utils.run_bass_kernel_spmd(
        nc, in_maps, core_ids=list(range(NCORES)), trace=TRACE
    )
    LAST_EXEC_NS = res.exec_time_ns
    LAST_TRACE = res.instructions_and_trace[1] if res.instructions_and_trace else None

    y = np.empty((B, C, FLAT), np.float32)
    if use_sparse:
        for cid in range(NCORES):
            for s in range(BPC):
                y[int(slots[s, cid])] = res.results[cid]["out"][s]
    else:
        for cid in range(NCORES):
            y[cid * BPC : (cid + 1) * BPC] = res.results[cid]["out"]
    return np.ascontiguousarray(
        y.reshape(B, C, PH, PW)[:, :, 1 : H + 1, 1 : W + 1]
    )


# revision 22
# speedup vs baseline: 1.3965x; 1.0203x over previous
"""Trainium2 Bass kernel for the sparse BasicBlock problem.

Math (masks and `vector` are binary in setup_inputs; verified at runtime):
    g   = x * mask_dilate
    c1  = conv3x3(g, w1)
    h   = relu(c1 * s1v + t1v) * mask      (BN1 affine folded with `vector`)
    c2  = conv3x3(h, w2)
    out = relu(x + (c2 * s2 + t2) * mask)

Layout: per image, channels on SBUF partitions, spatial flattened as a
zero-padded (H+2)x(W+2) row-major plane so a 3x3 conv is 9 shifted matmuls
accumulating in PSUM (shift = (dy-1)*(W+2) + (dx-1)). Matmuls run in fp32r
(full PE rate, ~1e-4 relative error).

Channel sparsity: `vector` zeroes ~half of conv1's output channels per image
(h == 0 there), so conv1 computes only the active channels (M-compaction) and
conv2 contracts only over them (K-compaction), via host-side gathered and
zero-padded per-image weights. One SPMD program is shared by all 8 cores, so
images are sorted by active-channel count and assigned so that each image
slot has a fixed channel-tile count across cores (max over cores).

Sharding: data-parallel over batch, 4 images per core on 8 cores.
"""

import sys
import types
from contextlib import ExitStack

sys.path.insert(0, "/opt/trn_rl_repo")

import ml_dtypes
import numpy as np

import concourse.bacc as bacc
import concourse.bass as bass
import concourse.mybir as mybir
import concourse.tile as tile
from concourse import bass_utils

# ----------------------------------------------------------------------------
# axon NTFF profiling hook shim (enables trace=True under axon)
# ----------------------------------------------------------------------------
_HOOK = {"hook": None}


def _install_axon_hooks():
    try:
        import antenv  # noqa: F401
    except ImportError:
        return
    if "antenv.axon_hooks" not in sys.modules:
        mod = types.ModuleType("antenv.axon_hooks")
        mod.set_axon_ntff_profile_hook = lambda h: _HOOK.__setitem__("hook", h)
        mod.get_axon_ntff_profile_hook = lambda: _HOOK["hook"]
        sys.modules["antenv.axon_hooks"] = mod
    if _HOOK["hook"] is None:
        try:
            from trn_agent_boot.trn_boot import _ntff_profile_via_ctypes

            sys.modules["antenv.axon_hooks"].set_axon_ntff_profile_hook(
                _ntff_profile_via_ctypes("/opt/axon/libaxon_pjrt.so")
            )
        except Exception:
            pass


_install_axon_hooks()
bass_utils.upload_artifacts = lambda tmpdir: tmpdir  # no S3 in this container

# ----------------------------------------------------------------------------
# problem constants (hardcoded per spec)
# ----------------------------------------------------------------------------
B, C, H, W = 32, 256, 56, 56
NCORES = 8
BPC = B // NCORES
EPS = 1e-5

TRACE = False
MM_MODE = "f32r"  # 'f32r' | 'bf16' | 'f32'
SPARSE = True
LAST_EXEC_NS = None
LAST_TRACE = None

F32 = mybir.dt.float32
BF16 = mybir.dt.bfloat16


def _chunks(total, maxw):
    """EVEN-width chunks <= maxw (fp32r needs an even moving dim; >=256 keeps
    full PE rate)."""
    assert total % 2 == 0, total
    n = -(-total // maxw)
    base = (total // n) & ~1
    rem = total - base * n
    out = []
    off = 0
    for i in range(n):
        w = base + (2 if i < rem // 2 else 0)
        out.append((off, w))
        off += w
    assert off == total
    return out


def _mdt(mm_mode):
    return {"f32r": mybir.dt.float32r, "bf16": BF16, "f32": F32}[mm_mode]


def build_nc(mm_mode=MM_MODE, bpc=BPC, c=C, h=H, w=W, slot_tiles=None):
    """Build the per-core SPMD Bass program.

    slot_tiles: None for the dense kernel, else per-image-slot channel-tile
    counts (e.g. (2, 2, 1, 1)) for the sparsity-specialized kernel.
    """
    PW, PH = w + 2, h + 2
    FLAT = PH * PW
    CT = c // 128
    NS = 9
    shifts = [(dy - 1) * PW + (dx - 1) for dy in range(3) for dx in range(3)]
    out_lo = PW + 1
    out_hi = h * PW + w
    span = out_hi - out_lo + 1
    chunks = [(out_lo + o, s) for (o, s) in _chunks(span, 464)]
    chunk_alloc = max(s for _, s in chunks)

    sparse = slot_tiles is not None
    if sparse:
        assert len(slot_tiles) == bpc
        max_nt = max(slot_tiles)
    mdt = _mdt(mm_mode)
    edt = F32 if mm_mode == "f32r" else mdt

    nc = bacc.Bacc("TRN2", debug=False, enable_asserts=False, num_devices=NCORES)

    # x / masks / out are passed HOST-PADDED to the (h+2)x(w+2) plane so every
    # large DMA is fully contiguous
    x_d = nc.dram_tensor("x", [bpc, c, FLAT], F32, kind="ExternalInput").ap()
    mask_d = nc.dram_tensor("mask", [bpc, FLAT], BF16, kind="ExternalInput").ap()
    maskd_d = nc.dram_tensor("maskd", [bpc, FLAT], BF16, kind="ExternalInput").ap()
    s2_d = nc.dram_tensor("s2", [c, 1], F32, kind="ExternalInput").ap()
    t2_d = nc.dram_tensor("t2", [c, 1], F32, kind="ExternalInput").ap()
    out_d = nc.dram_tensor("out", [bpc, c, FLAT], F32, kind="ExternalOutput").ap()
    if sparse:
        w1_d, w2_d, s1_d, t1_d = [], [], [], []
        for s, nt in enumerate(slot_tiles):
            np_s = 128 * nt
            w1_d.append(
                nc.dram_tensor(f"w1g{s}", [CT, 128, NS, np_s], mdt, kind="ExternalInput").ap()
            )
            w2_d.append(
                nc.dram_tensor(f"w2g{s}", [nt, 128, NS, c], mdt, kind="ExternalInput").ap()
            )
            s1_d.append(
                nc.dram_tensor(f"s1vg{s}", [np_s, 1], F32, kind="ExternalInput").ap()
            )
            t1_d.append(
                nc.dram_tensor(f"t1vg{s}", [np_s, 1], F32, kind="ExternalInput").ap()
            )
    else:
        w1s_d = nc.dram_tensor("w1", [CT, 128, NS, c], mdt, kind="ExternalInput").ap()
        w2s_d = nc.dram_tensor("w2", [CT, 128, NS, c], mdt, kind="ExternalInput").ap()
        s1v_d = nc.dram_tensor("s1v", [bpc, c, 1], F32, kind="ExternalInput").ap()
        t1v_d = nc.dram_tensor("t1v", [bpc, c, 1], F32, kind="ExternalInput").ap()

    Relu = mybir.ActivationFunctionType.Relu
    Ident = mybir.ActivationFunctionType.Identity

    with tile.TileContext(nc) as tc, ExitStack() as ctx:
        wpool = ctx.enter_context(tc.tile_pool(name="wpool", bufs=1))
        w1pool = ctx.enter_context(tc.tile_pool(name="w1pool", bufs=2))
        cpool = ctx.enter_context(tc.tile_pool(name="cpool", bufs=1))
        ppool = ctx.enter_context(tc.tile_pool(name="ppool", bufs=2))
        xpool = ctx.enter_context(tc.tile_pool(name="xpool", bufs=CT + 1))
        spool = ctx.enter_context(tc.tile_pool(name="spool", bufs=CT + 1))
        hpool = ctx.enter_context(
            tc.tile_pool(name="hpool", bufs=(max(2, max_nt) if sparse else CT))
        )
        mpool = ctx.enter_context(tc.tile_pool(name="mpool", bufs=2))
        mdpool = ctx.enter_context(tc.tile_pool(name="mdpool", bufs=2))
        epool = ctx.enter_context(tc.tile_pool(name="epool", bufs=8))
        pspool = ctx.enter_context(tc.tile_pool(name="psum", bufs=8, space="PSUM"))

        # bn2 params (shared)
        s2_sb = cpool.tile([128, CT, 1], F32)
        t2_sb = cpool.tile([128, CT, 1], F32)
        for co_t in range(CT):
            nc.scalar.dma_start(out=s2_sb[:, co_t], in_=s2_d[co_t * 128 : (co_t + 1) * 128])
            nc.scalar.dma_start(out=t2_sb[:, co_t], in_=t2_d[co_t * 128 : (co_t + 1) * 128])

        if not sparse:
            w1_sb = wpool.tile([128, CT, NS, c], mdt)
            w2_sb = wpool.tile([128, CT, NS, c], mdt)

        for i in range(bpc):
            nt = slot_tiles[i] if sparse else CT  # conv1 output tiles / conv2 K tiles
            np_i = 128 * nt

            # ---- masks: 1-row DMA into partition 0, then in-place broadcast ----
            # image 0 is latency-critical: split the x DMA / maskd broadcast /
            # g multiply into half-planes so the first chunk-group's matmuls
            # (which only read the low half) start as early as possible
            if i == 0 and len(chunks) >= 2:
                ga_off, ga_wd = chunks[len(chunks) // 2 - 1]
                hb = ga_off + ga_wd + out_lo  # last read of chunk-group A
                halves = [(0, hb), (hb, FLAT)]
            else:
                halves = [(0, FLAT)]
            maskd_pad = mdpool.tile([128, FLAT], BF16, tag="md", name=f"maskd{i}")
            nc.sync.dma_start(out=maskd_pad[0:1, :], in_=maskd_d[i : i + 1])
            for lo, hi in halves:
                nc.gpsimd.partition_broadcast(
                    maskd_pad[:, lo:hi], maskd_pad[0:1, lo:hi]
                )

            mask_pad = mpool.tile([128, FLAT], BF16, tag="m", name=f"mask{i}")
            nc.sync.dma_start(out=mask_pad[0:1, :], in_=mask_d[i : i + 1])

            # ---- x (padded, sync ring) and g = x * mask_dilate ----
            x_pad, g_pad = [], []
            for ci_t in range(CT):
                xt = xpool.tile([128, FLAT], F32, tag="x", name=f"x{i}_{ci_t}")
                gt = spool.tile([128, FLAT], mdt, tag="scr", name=f"g{i}_{ci_t}")
                for lo, hi in halves:
                    nc.sync.dma_start(
                        out=xt[:, lo:hi], in_=x_d[i, ci_t * 128 : (ci_t + 1) * 128][:, lo:hi]
                    )
                    nc.vector.tensor_mul(gt[:, lo:hi], xt[:, lo:hi], maskd_pad[:, lo:hi])
                x_pad.append(xt)
                g_pad.append(gt)
            nc.gpsimd.partition_broadcast(mask_pad, mask_pad[0:1, :])

            # ---- weights for this image (scalar/HWDGE ring) ----
            if sparse:
                w1_sb = w1pool.tile([128, CT, NS, np_i], mdt, tag="w1g", name=f"w1g{i}")
                for ci_t in range(CT):
                    nc.scalar.dma_start(out=w1_sb[:, ci_t], in_=w1_d[i][ci_t])
                w2_sb = wpool.tile([128, nt, NS, c], mdt, tag="w2g", name=f"w2g{i}")
                for ci_t in range(nt):
                    nc.scalar.dma_start(out=w2_sb[:, ci_t], in_=w2_d[i][ci_t])
            elif i == 0:
                for ci_t in range(CT):
                    nc.scalar.dma_start(out=w1_sb[:, ci_t], in_=w1s_d[ci_t])
                    nc.scalar.dma_start(out=w2_sb[:, ci_t], in_=w2s_d[ci_t])

            # ---- folded bn1*vector params ----
            s1v_t = ppool.tile([128, nt, 1], F32, tag="s1v", name=f"s1v{i}")
            t1v_t = ppool.tile([128, nt, 1], F32, tag="t1v", name=f"t1v{i}")
            for co_t in range(nt):
                if sparse:
                    nc.scalar.dma_start(
                        out=s1v_t[:, co_t], in_=s1_d[i][co_t * 128 : (co_t + 1) * 128]
                    )
                    nc.scalar.dma_start(
                        out=t1v_t[:, co_t], in_=t1_d[i][co_t * 128 : (co_t + 1) * 128]
                    )
                else:
                    nc.scalar.dma_start(
                        out=s1v_t[:, co_t], in_=s1v_d[i, co_t * 128 : (co_t + 1) * 128]
                    )
                    nc.scalar.dma_start(
                        out=t1v_t[:, co_t], in_=t1v_d[i, co_t * 128 : (co_t + 1) * 128]
                    )

            # ---- conv1 -> h (active channels only in sparse mode) ----
            h_pad = []
            for co_t in range(nt):
                ht = hpool.tile([128, FLAT], mdt, tag="h", name=f"h{i}_{co_t}")
                nc.vector.tensor_scalar_mul(ht[:, 0:out_lo], x_pad[0][:, 0:out_lo], 0.0)
                nc.vector.tensor_scalar_mul(
                    ht[:, out_hi + 1 : FLAT], x_pad[0][:, out_hi + 1 : FLAT], 0.0
                )
                h_pad.append(ht)

            def epi1(co_t, off, wd, ps):
                r = epool.tile([128, chunk_alloc], edt, tag="e", name=f"r{i}_{co_t}_{off}")
                nc.scalar.activation(
                    r[:, :wd], ps[:, :wd], Relu, bias=t1v_t[:, co_t], scale=s1v_t[:, co_t]
                )
                nc.vector.tensor_mul(
                    h_pad[co_t][:, off : off + wd], r[:, :wd], mask_pad[:, off : off + wd]
                )

            # weight-stationary grouped accumulation: per co-tile, chunks are
            # processed in 2 groups; within a group the (ci,shift) loop is
            # outer so each weight tile is loaded once per group, and group A's
            # epilogues overlap group B's matmuls
            def grouped_conv(w_sb, n_k, rhs, n_out, epi, pfx):
                groups = [
                    list(enumerate(chunks))[0 : len(chunks) // 2],
                    list(enumerate(chunks))[len(chunks) // 2 :],
                ]
                for co_t in range(n_out):
                    for gi, grp in enumerate(groups):
                        pss = {
                            ck: pspool.tile(
                                [128, chunk_alloc], F32, tag="ps", name=f"{pfx}_{co_t}_{ck}"
                            )
                            for ck, _ in grp
                        }
                        k, nk = 0, n_k * NS
                        for ci_t in range(n_k):
                            for s in range(NS):
                                for ck, (off, wd) in grp:
                                    nc.tensor.matmul(
                                        pss[ck][:, :wd],
                                        w_sb[:, ci_t, s, co_t * 128 : co_t * 128 + 128],
                                        rhs[ci_t][:, off + shifts[s] : off + shifts[s] + wd],
                                        start=(k == 0),
                                        stop=(k == nk - 1),
                                    )
                                k += 1
                        for ck, (off, wd) in grp:
                            epi(co_t, off, wd, pss[ck])

            def epi1(co_t, off, wd, ps):
                r = epool.tile([128, chunk_alloc], edt, tag="e", name=f"r{i}_{co_t}_{off}")
                nc.scalar.activation(
                    r[:, :wd], ps[:, :wd], Relu, bias=t1v_t[:, co_t], scale=s1v_t[:, co_t]
                )
                nc.vector.tensor_mul(
                    h_pad[co_t][:, off : off + wd], r[:, :wd], mask_pad[:, off : off + wd]
                )

            grouped_conv(w1_sb, CT, g_pad, nt, epi1, f"ps1_{i}")

            # ---- conv2 -> out ----
            out_t = []
            for ct in range(CT):
                ot = spool.tile([128, FLAT], F32, tag="scr", name=f"o{i}_{ct}")
                nc.vector.memset(ot[:, 0:out_lo], 0.0)
                nc.vector.memset(ot[:, out_hi + 1 : FLAT], 0.0)
                out_t.append(ot)
            def epi2(co_t, off, wd, ps):
                e = epool.tile([128, chunk_alloc], F32, tag="e", name=f"e{i}_{co_t}_{off}")
                nc.scalar.activation(
                    e[:, :wd], ps[:, :wd], Ident, bias=t2_sb[:, co_t], scale=s2_sb[:, co_t]
                )
                nc.vector.tensor_mul(e[:, :wd], e[:, :wd], mask_pad[:, off : off + wd])
                dst = out_t[co_t][:, off : off + wd]
                nc.vector.tensor_add(dst, e[:, :wd], x_pad[co_t][:, off : off + wd])
                nc.scalar.activation(dst, dst, Relu)

            grouped_conv(w2_sb, nt, h_pad, CT, epi2, f"ps2_{i}")

            osplit = chunks[len(chunks) // 2][0]  # group A covers [0, osplit)
            for co_t in range(CT):
                eng = nc.sync if co_t == 0 else nc.scalar
                eng.dma_start(
                    out=out_d[i, co_t * 128 : (co_t + 1) * 128][:, 0:osplit],
                    in_=out_t[co_t][:, 0:osplit],
                )
                eng.dma_start(
                    out=out_d[i, co_t * 128 : (co_t + 1) * 128][:, osplit:FLAT],
                    in_=out_t[co_t][:, osplit:FLAT],
                )

    nc.compile()
    return nc


# ----------------------------------------------------------------------------
# host-side prep + execution
# ----------------------------------------------------------------------------
_NC_CACHE = {}


def _get_nc(key, **kw):
    if key not in _NC_CACHE:
        _NC_CACHE[key] = build_nc(**kw)
    return _NC_CACHE[key]


def _wt_np(mm_mode):
    return ml_dtypes.bfloat16 if mm_mode == "bf16" else np.float32


def _prep_weights(wt, mm_mode, c=C):
    # [co, ci, 3, 3] -> [ci_t, ci, s, co] with s = dy*3+dx
    t = np.ascontiguousarray(wt.transpose(1, 2, 3, 0).reshape(c // 128, 128, 9, c))
    return t.astype(_wt_np(mm_mode))


def kernel(**inputs):
    global LAST_EXEC_NS, LAST_TRACE
    x = np.asarray(inputs["x"], dtype=np.float32)
    mask = np.asarray(inputs["mask"], dtype=np.float32).reshape(B, H * W)
    maskd = np.asarray(inputs["mask_dilate"], dtype=np.float32).reshape(B, H * W)
    vector = np.asarray(inputs["vector"], dtype=np.float32)
    w1 = np.asarray(inputs["conv1_w"], dtype=np.float32)
    w2 = np.asarray(inputs["conv2_w"], dtype=np.float32)

    s1 = np.asarray(inputs["bn1_g"]) / np.sqrt(np.asarray(inputs["bn1_v"]) + EPS)
    t1 = np.asarray(inputs["bn1_b"]) - np.asarray(inputs["bn1_m"]) * s1
    s2 = np.asarray(inputs["bn2_g"]) / np.sqrt(np.asarray(inputs["bn2_v"]) + EPS)
    t2 = np.asarray(inputs["bn2_b"]) - np.asarray(inputs["bn2_m"]) * s2
    s1, t1 = s1.astype(np.float32), t1.astype(np.float32)

    binary = lambda a: bool(np.isin(a, (0.0, 1.0)).all())  # noqa: E731
    masks_binary = binary(mask) and binary(maskd)
    assert (vector >= 0).all() and masks_binary, (
        "kernel specialized for setup_inputs-style binary masks / nonneg vector"
    )
    use_sparse = SPARSE and binary(vector)

    # host-pad x and masks to the (H+2)x(W+2) plane => contiguous device DMAs
    PW, PH = W + 2, H + 2
    FLAT = PH * PW
    xp = np.zeros((B, C, PH, PW), np.float32)
    xp[:, :, 1 : H + 1, 1 : W + 1] = x
    xp = xp.reshape(B, C, FLAT)
    mask_bf = np.zeros((B, PH, PW), ml_dtypes.bfloat16)
    mask_bf[:, 1 : H + 1, 1 : W + 1] = mask.reshape(B, H, W)
    mask_bf = mask_bf.reshape(B, FLAT)
    maskd_bf = np.zeros((B, PH, PW), ml_dtypes.bfloat16)
    maskd_bf[:, 1 : H + 1, 1 : W + 1] = maskd.reshape(B, H, W)
    maskd_bf = maskd_bf.reshape(B, FLAT)
    mm_mode = MM_MODE
    wdt = _wt_np(mm_mode)

    if use_sparse:
        nact = vector.sum(1).astype(int)
        order = np.argsort(-nact, kind="stable")
        slots = order.reshape(BPC, NCORES)  # [slot, core] -> original image idx
        # put a cheap (low tile-count) slot first so image 0's setup is light,
        # then the heavy slots
        rank = np.argsort([nact[slots[s]].max() for s in range(BPC)])
        perm = []
        light, heavy = list(rank), []
        if BPC >= 2:
            light, heavy = [rank[0]], list(rank[1:][::-1])
        perm = light + heavy
        slots = slots[perm]
        slot_tiles = tuple(
            max(1, int(np.ceil(nact[slots[s]].max() / 128))) for s in range(BPC)
        )
        if sum(slot_tiles) >= BPC * (C // 128):
            use_sparse = False  # no win; fall back to shared-weight dense kernel

    if use_sparse:
        nc = _get_nc(("sparse", mm_mode, slot_tiles), mm_mode=mm_mode, slot_tiles=slot_tiles)
        # full lhsT layouts to gather from
        w1l = w1.transpose(1, 2, 3, 0).reshape(C, 9, C)  # [ci, s, co]
        w2r = w2.transpose(1, 2, 3, 0).reshape(C, 9, C)  # [ci, s, co] rows = conv2 input ch
        in_maps = []
        for cid in range(NCORES):
            imgs = [int(slots[s, cid]) for s in range(BPC)]
            m = dict(
                x=np.ascontiguousarray(xp[imgs]),
                mask=np.ascontiguousarray(mask_bf[imgs]),
                maskd=np.ascontiguousarray(maskd_bf[imgs]),
                s2=np.ascontiguousarray(s2.reshape(C, 1).astype(np.float32)),
                t2=np.ascontiguousarray(t2.reshape(C, 1).astype(np.float32)),
            )
            for s, b in enumerate(imgs):
                np_s = 128 * slot_tiles[s]
                idx = np.where(vector[b] > 0)[0]
                k = len(idx)
                idxp = np.zeros(np_s, dtype=int)
                idxp[:k] = idx
                # conv1 weights gathered on OUTPUT channel; pad -> zero
                w1g = w1l[:, :, idxp].copy()  # [ci, s, np_s]
                w1g[:, :, k:] = 0
                m[f"w1g{s}"] = np.ascontiguousarray(
                    w1g.reshape(C // 128, 128, 9, np_s)
                ).astype(wdt)
                # conv2 weights gathered on INPUT channel; pad -> zero
                w2g = w2r[idxp].copy()  # [np_s, s, co]
                w2g[k:] = 0
                m[f"w2g{s}"] = np.ascontiguousarray(
                    w2g.reshape(slot_tiles[s], 128, 9, C)
                ).astype(wdt)
                sg = np.zeros(np_s, np.float32)
                tg = np.zeros(np_s, np.float32)
                sg[:k] = s1[idx]
                tg[:k] = t1[idx]
                m[f"s1vg{s}"] = sg.reshape(np_s, 1)
                m[f"t1vg{s}"] = tg.reshape(np_s, 1)
            in_maps.append(m)
    else:
        nc = _get_nc(("dense", mm_mode), mm_mode=mm_mode)
        s1v = (s1[None, :] * vector).astype(np.float32)
        t1v = (t1[None, :] * vector).astype(np.float32)
        w1l = _prep_weights(w1, mm_mode)
        w2l = _prep_weights(w2, mm_mode)
        xs = xp.reshape(NCORES, BPC, C, FLAT)
        in_maps = []
        for cid in range(NCORES):
            sl = slice(cid * BPC, (cid + 1) * BPC)
            in_maps.append(
                dict(
                    x=np.ascontiguousarray(xs[cid]),
                    mask=np.ascontiguousarray(mask_bf[sl]),
                    maskd=np.ascontiguousarray(maskd_bf[sl]),
                    w1=w1l,
                    w2=w2l,
                    s1v=np.ascontiguousarray(s1v[sl].reshape(BPC, C, 1)),
                    t1v=np.ascontiguousarray(t1v[sl].reshape(BPC, C, 1)),
                    s2=np.ascontiguousarray(s2.reshape(C, 1).astype(np.float32)),
                    t2=np.ascontiguousarray(t2.reshape(C, 1).astype(np.float32)),
                )
            )

    res = bass_utils.run_bass_kernel_spmd(
        nc, in_maps, core_ids=list(range(NCORES)), trace=TRACE
    )
    LAST_EXEC_NS = res.exec_time_ns
    LAST_TRACE = res.instructions_and_trace[1] if res.instructions_and_trace else None

    y = np.empty((B, C, FLAT), np.float32)
    if use_sparse:
        for cid in range(NCORES):
            for s in range(BPC):
                y[int(slots[s, cid])] = res.results[cid]["out"][s]
    else:
        for cid in range(NCORES):
            y[cid * BPC : (cid + 1) * BPC] = res.results[cid]["out"]
    return np.ascontiguousarray(
        y.reshape(B, C, PH, PW)[:, :, 1 : H + 1, 1 : W + 1]
    )
